# revision 14
# baseline (speedup 1.0000x reference)
"""Brownian-bridge criterion loss on 8 Trainium2 NeuronCores (3 launches).

Data-parallel over the 1600 pivot-sorted cur sequences (200/core); the
negative pool is column-sharded (each core scores all 1664 row-slots against
its own 400 sequences' pivot frames). Host work between launches is pure
indexing (concat / transpose / gather).

Launch 1 (value-independent): transposed projection ([hid, cols] inputs,
  W stationary) of the 400 seqs' interior frames; column norms via all-ones
  [128,128] stationary matmul (sums broadcast across PSUM partitions) +
  1/sqrt on scalar engine; bias+normalize fused into PSUM evacuation ->
  pool [c, 14, 400] bf16. Small projection of cur head/pivot/tail columns
  -> a-vectors [c, 200], per-row dots via elementwise+ones-matmul, per-row
  scalars (c0, s, softplus) packed [200, 4].

Launch 2 (specialized to the bridge pivot multiset): cross = A^T @ pool_g
  per (row-tile, pivot-run) segment, Max8 per tile -> top8 [128, 13, 8].

Launch 3 (value-independent): merge the 8 cores' top8 (host-gathered into
  [128, 8, 13, 8]), dist = c1*cross + c0, top-8 of 64, exp /
  top-5-excluding-self trick, masked mean -> (brownian, head_tail).
"""

import sys

sys.path.insert(0, "/opt/trn_rl_repo")

import numpy as np
import ml_dtypes

import concourse.bacc as bacc
import concourse.bass as bass
import concourse.mybir as mybir
import concourse.tile as tile
from concourse.bass_utils import run_bass_kernel_spmd

F32 = mybir.dt.float32
BF16 = mybir.dt.bfloat16
I32 = mybir.dt.int32
AF = mybir.ActivationFunctionType
OP = mybir.AluOpType

BS, T, Q, HID, PROJ = 16, 16, 100, 256, 256
NSEQ = BS * Q              # 1600 positive sequences
NCORES = 8
SPC = NSEQ // NCORES       # 200 cur sequences per core
NG = T - 2                 # 14 interior pivot positions (1..14)
NCOL = 2 * SPC             # 400 negative-pool columns per core
NT = (NSEQ + 127) // 128   # 13 row tiles
NPAD = NT * 128            # 1664
DELTA = 0.3
GSZ = 3 * SPC              # 600 head/pivot/tail columns
NT3 = 16                   # fixed slot-tile count for launch 3 (2048 slots)


def _build_l1():
    nc = bacc.Bacc("TRN2", target_bir_lowering=False, debug=False,
                   num_devices=NCORES)
    xt_int = nc.declare_dram_parameter("xt_int", [2, 128, NG, NCOL], BF16,
                                       isOutput=False)
    xt_g = nc.declare_dram_parameter("xt_g", [2, 128, GSZ], BF16,
                                     isOutput=False)
    w_in = nc.declare_dram_parameter("w_in", [2, 128, PROJ], BF16,
                                     isOutput=False)
    bc_in = nc.declare_dram_parameter("bc_in", [128, 2], F32, isOutput=False)
    # [1,200] host scalars: (1-alpha), alpha, -1/(2 sigma^2)
    hrow_in = nc.declare_dram_parameter("hrow_in", [1, 3 * SPC], F32,
                                        isOutput=False)
    pool_out = nc.declare_dram_parameter("pool_out", [2, 128, NG, NCOL], BF16,
                                         isOutput=True)
    a_out = nc.declare_dram_parameter("a_out", [2, 128, SPC], BF16,
                                      isOutput=True)
    sc_out = nc.declare_dram_parameter("sc_out", [1, SPC * 4], F32,
                                       isOutput=True)

    with tile.TileContext(nc) as tc:
        with (
            tc.tile_pool(name="singles", bufs=1) as singles,
            tc.tile_pool(name="work", bufs=4) as work,
            tc.tile_pool(name="psA", bufs=4, space="PSUM") as psA,
            tc.tile_pool(name="psB", bufs=3, space="PSUM") as psB,
        ):
            # ---- input loads: W/g-cols first (g-phase runs first), then
            # the big interior chunks split across sync/gpsimd queues ----
            w_sb = []
            for kh in range(2):
                t_w = singles.tile([128, PROJ], BF16, tag=f"w{kh}")
                nc.scalar.dma_start(out=t_w, in_=w_in[kh])
                w_sb.append(t_w)
            bias_sb = singles.tile([128, 2], F32, tag="bias")
            nc.scalar.dma_start(out=bias_sb, in_=bc_in[:, :])
            hrow_sb = singles.tile([1, 3 * SPC], F32, tag="hrow")
            nc.scalar.dma_start(out=hrow_sb, in_=hrow_in[:, :])
            h_oma = hrow_sb[:, 0:SPC]
            h_alp = hrow_sb[:, SPC:2 * SPC]
            h_c1hneg = hrow_sb[:, 2 * SPC:3 * SPC]
            xtg_sb = []
            for kh in range(2):
                t_g = singles.tile([128, GSZ], BF16, tag=f"xtg{kh}")
                eng = nc.sync if kh == 0 else nc.gpsimd
                eng.dma_start(out=t_g, in_=xt_g[kh])
                xtg_sb.append(t_g)
            NCHK = 2
            GPC = NG // NCHK  # 7
            xt_sb = [[None] * NCHK for _ in range(2)]
            for ck in range(NCHK):
                for kh in range(2):
                    t_x = singles.tile([128, GPC, NCOL], BF16,
                                       tag=f"xt{kh}_{ck}")
                    eng = nc.sync if kh == 0 else nc.gpsimd
                    eng.dma_start(
                        out=t_x, in_=xt_int[kh, :, ck * GPC:(ck + 1) * GPC, :])
                    xt_sb[kh][ck] = t_x

            ones_sb = singles.tile([128, 128], BF16, tag="ones")
            nc.vector.memset(ones_sb, 1.0)
            onesrow = singles.tile([1, 128], BF16, tag="onesrow")
            nc.vector.memset(onesrow, 1.0)

            pool_sb = [singles.tile([128, NG, NCOL], BF16, tag=f"pool{kh}",
                                    name=f"pool{kh}")
                       for kh in range(2)]

            # ------------- head/pivot/tail projection + per-row scalars ----
            # emitted FIRST: its small PE work runs up front and its DVE /
            # scalar tail overlaps the main projection loop below.
            u_g = [work.tile([128, GSZ], F32, tag=f"ug{mh}", name=f"ug{mh}")
                   for mh in range(2)]
            xsq_g = [work.tile([128, GSZ], BF16, tag=f"xsqg{mh}",
                              name=f"xsqg{mh}") for mh in range(2)]
            for mh in range(2):
                for c2 in range(2):
                    sl = slice(c2 * 300, (c2 + 1) * 300)
                    p = psA.tile([128, 300], F32, tag="p")
                    for kh in range(2):
                        nc.tensor.matmul(
                            out=p,
                            lhsT=w_sb[kh][:, mh * 128:(mh + 1) * 128],
                            rhs=xtg_sb[kh][:, sl],
                            start=(kh == 0), stop=(kh == 1))
                    nc.vector.tensor_scalar(
                        out=u_g[mh][:, sl], in0=p,
                        scalar1=bias_sb[:, mh:mh + 1], scalar2=None,
                        op0=OP.add)
                    nc.scalar.activation(out=xsq_g[mh][:, sl], in_=p,
                                         func=AF.Square,
                                         bias=bias_sb[:, mh:mh + 1])
            rinv_g = []
            for j in range(3):
                gn = psB.tile([1, SPC], F32, name=f"gn{j}", tag="acc")
                for mh in range(2):
                    nc.tensor.matmul(
                        out=gn, lhsT=ones_sb[:, 0:1],
                        rhs=xsq_g[mh][:, j * SPC:(j + 1) * SPC],
                        start=(mh == 0), stop=(mh == 1))
                rv = work.tile([1, SPC], F32, tag=f"rinvg{j}",
                               name=f"rinvg{j}")
                nc.scalar.activation(out=rv, in_=gn,
                                     func=AF.Abs_reciprocal_sqrt)
                rinv_g.append(rv)

            # a = (1-alpha)*u0/||u0|| + alpha*u2/||u2||
            s02 = work.tile([1, 2 * SPC], BF16, tag="s02")
            nc.vector.tensor_tensor(out=s02[:, 0:SPC], in0=h_oma,
                                    in1=rinv_g[0], op=OP.mult)
            nc.vector.tensor_tensor(out=s02[:, SPC:2 * SPC], in0=h_alp,
                                    in1=rinv_g[2], op=OP.mult)
            ps_s = psA.tile([128, 2 * SPC], F32, tag="pss", bufs=1)
            nc.tensor.matmul(out=ps_s, lhsT=onesrow, rhs=s02,
                             start=True, stop=True)
            a_f = [work.tile([128, SPC], F32, tag=f"af{mh}", name=f"af{mh}")
                   for mh in range(2)]
            a_bf = [work.tile([128, SPC], BF16, tag=f"abf{mh}", name=f"abf{mh}")
                    for mh in range(2)]
            pst = [work.tile([128, 3, SPC], BF16, tag=f"pst{mh}", name=f"pst{mh}")
                   for mh in range(2)]
            for mh in range(2):
                u0 = u_g[mh][:, 0:SPC]
                u1 = u_g[mh][:, SPC:2 * SPC]
                u2 = u_g[mh][:, 2 * SPC:3 * SPC]
                t1 = work.tile([128, SPC], F32, tag="t1")
                nc.vector.tensor_tensor(out=t1, in0=u0, in1=ps_s[:, 0:SPC],
                                        op=OP.mult)
                t2 = work.tile([128, SPC], F32, tag="t2")
                nc.vector.tensor_tensor(out=t2, in0=u2,
                                        in1=ps_s[:, SPC:2 * SPC], op=OP.mult)
                nc.vector.tensor_tensor(out=a_f[mh], in0=t1, in1=t2, op=OP.add)
                nc.vector.tensor_copy(out=a_bf[mh], in_=a_f[mh])
                nc.gpsimd.dma_start(out=a_out[mh], in_=a_bf[mh])
                nc.vector.tensor_tensor(out=pst[mh][:, 0, :], in0=a_f[mh],
                                        in1=u1, op=OP.mult)
                nc.vector.tensor_tensor(out=pst[mh][:, 1, :], in0=a_f[mh],
                                        in1=a_f[mh], op=OP.mult)
                nc.vector.tensor_tensor(out=pst[mh][:, 2, :], in0=u0,
                                        in1=u2, op=OP.mult)
            pd = []
            for j in range(3):
                pdj = psB.tile([1, SPC], F32, name=f"pd{j}", tag="acc")
                for mh in range(2):
                    nc.tensor.matmul(
                        out=pdj, lhsT=ones_sb[:, 0:1],
                        rhs=pst[mh][:, j, :], start=(mh == 0), stop=(mh == 1))
                pd.append(pdj)

            # scalar math in [1,200]: c0, s, softplus packed interleaved
            sc_pack = work.tile([1, SPC, 4], F32, tag="scpack")
            nc.vector.memset(sc_pack[:, :, 3:4], 0.0)
            q_r = work.tile([1, SPC], F32, tag="qr")
            nc.vector.tensor_tensor(out=q_r, in0=pd[0],
                                    in1=rinv_g[1], op=OP.mult)
            sc_r = work.tile([1, SPC], F32, tag="scr")
            nc.vector.tensor_tensor(out=sc_r, in0=pd[2],
                                    in1=rinv_g[0], op=OP.mult)
            nc.vector.tensor_tensor(out=sc_r, in0=sc_r,
                                    in1=rinv_g[2], op=OP.mult)
            # c0 = (1 + aa) * (-1/(2 sigma^2))
            nc.vector.scalar_tensor_tensor(
                out=sc_pack[:, :, 0], in0=pd[1], scalar=1.0,
                in1=h_c1hneg, op0=OP.add, op1=OP.mult)
            # s = (1 - 2q + aa) * (-1/(2 sigma^2))
            t_r = work.tile([1, SPC], F32, tag="tr")
            nc.vector.scalar_tensor_tensor(
                out=t_r, in0=q_r, scalar=-2.0, in1=pd[1],
                op0=OP.mult, op1=OP.add)
            nc.vector.tensor_scalar(out=t_r, in0=t_r, scalar1=1.0,
                                    scalar2=None, op0=OP.add)
            nc.vector.tensor_tensor(out=sc_pack[:, :, 1], in0=t_r,
                                    in1=h_c1hneg, op=OP.mult)
            # softplus(delta - score)
            delta_sb = work.tile([1, 1], F32, tag="delta")
            nc.vector.memset(delta_sb, DELTA)
            e_r = work.tile([1, SPC], F32, tag="er")
            nc.scalar.activation(out=e_r, in_=sc_r, func=AF.Exp,
                                 scale=-1.0, bias=delta_sb)
            nc.scalar.activation(out=sc_pack[:, :, 2], in_=e_r, func=AF.Ln,
                                 bias=1.0)
            nc.gpsimd.dma_start(
                out=sc_out[:, :],
                in_=sc_pack[:, :, :].rearrange("o s q -> o (s q)"))

            # ------------- main projection + normalize, per group-pair -----
            # matmuls ordered (mh, kh) outer so the stationary W chunk is
            # loaded only 4x per pair; evacuation split across DVE/gpsimd.
            for gp in range(NG // 2):
                ps = [[None] * 2 for _ in range(2)]  # [g2][mh]
                for g2 in range(2):
                    for mh in range(2):
                        ps[g2][mh] = psA.tile([128, NCOL], F32, tag="p",
                                              name=f"p{g2}{mh}")
                for mh in range(2):
                    for kh in range(2):
                        for g2 in range(2):
                            g = 2 * gp + g2
                            ck, go = g // GPC, g % GPC
                            nc.tensor.matmul(
                                out=ps[g2][mh],
                                lhsT=w_sb[kh][:, mh * 128:(mh + 1) * 128],
                                rhs=xt_sb[kh][ck][:, go, :],
                                start=(kh == 0), stop=(kh == 1))
                for g2 in range(2):
                    g = 2 * gp + g2
                    nb = psB.tile([128, NCOL], F32, tag="acc")
                    for mh in range(2):
                        xsq = work.tile([128, NCOL], BF16, tag="xsq")
                        nc.scalar.activation(out=xsq, in_=ps[g2][mh],
                                             func=AF.Square,
                                             bias=bias_sb[:, mh:mh + 1])
                        nc.tensor.matmul(out=nb, lhsT=ones_sb, rhs=xsq,
                                         start=(mh == 0), stop=(mh == 1))
                    rinvb = work.tile([128, NCOL], BF16, tag="rinvb")
                    nc.scalar.activation(out=rinvb, in_=nb,
                                         func=AF.Abs_reciprocal_sqrt)
                    for mh in range(2):
                        nc.vector.scalar_tensor_tensor(
                            out=pool_sb[mh][:, g, :], in0=ps[g2][mh],
                            scalar=bias_sb[:, mh:mh + 1], in1=rinvb,
                            op0=OP.add, op1=OP.mult)
                # normalized pool streamed out in two halves per side
                if gp == 3:
                    for mh in range(2):
                        eng = nc.sync if mh == 0 else nc.gpsimd
                        eng.dma_start(out=pool_out[mh, :, 0:8, :],
                                      in_=pool_sb[mh][:, 0:8, :])
            for mh in range(2):
                eng = nc.sync if mh == 0 else nc.gpsimd
                eng.dma_start(out=pool_out[mh, :, 8:NG, :],
                              in_=pool_sb[mh][:, 8:NG, :])
    nc.compile()
    return nc


def _build_l2(segments, nt2):
    """segments: tuple of (tile m, p0, p1, pivot g); p0/p1 32-aligned so
    every sub-matmul lands on a valid PE tile position."""
    nc = bacc.Bacc("TRN2", target_bir_lowering=False, debug=False,
                   num_devices=NCORES)
    a_in = nc.declare_dram_parameter("a_in", [2, 128, nt2 * 128], BF16,
                                     isOutput=False)
    pool_in = nc.declare_dram_parameter("pool_in", [2, 128, NG, NCOL], BF16,
                                        isOutput=False)
    t8_out = nc.declare_dram_parameter("t8_out", [128, nt2 * 8], F32,
                                       isOutput=True)

    with tile.TileContext(nc) as tc:
        with (
            tc.tile_pool(name="singles", bufs=1) as singles,
            tc.tile_pool(name="psA", bufs=6, space="PSUM") as psA,
        ):
            A_sb = []
            for kh in range(2):
                t_a = singles.tile([128, nt2 * 128], BF16, tag=f"A{kh}")
                eng = nc.sync if kh == 0 else nc.scalar
                eng.dma_start(out=t_a, in_=a_in[kh])
                A_sb.append(t_a)
            # pool: 2 chunks of 7 groups per half, in group order
            pool_sb = [singles.tile([128, NG, NCOL], BF16, tag=f"pool{kh}",
                                    name=f"pool{kh}")
                       for kh in range(2)]
            for ck in range(2):
                gsl = slice(ck * 7, (ck + 1) * 7)
                for kh in range(2):
                    eng = nc.sync if kh == 0 else nc.scalar
                    eng.dma_start(out=pool_sb[kh][:, gsl, :],
                                  in_=pool_in[kh, :, gsl, :])

            t8_sb = singles.tile([128, nt2, 8], F32, tag="t8")
            for m in range(nt2):
                px = psA.tile([128, NCOL], F32, tag="px")
                for (sm, p0, p1, g) in segments:
                    if sm != m:
                        continue
                    for kh in range(2):
                        nc.tensor.matmul(
                            out=px[p0:p1, :],
                            lhsT=A_sb[kh][:, m * 128 + p0:m * 128 + p1],
                            rhs=pool_sb[kh][:, g - 1, :],
                            start=(kh == 0), stop=(kh == 1))
                nc.vector.max(out=t8_sb[:, m, :], in_=px)
            nc.gpsimd.dma_start(
                out=t8_out[:, :],
                in_=t8_sb[:, :, :].rearrange("p t e -> p (t e)"))
    nc.compile()
    return nc


def _build_l3():
    nc = bacc.Bacc("TRN2", target_bir_lowering=False, debug=False,
                   num_devices=NCORES)
    cand_in = nc.declare_dram_parameter("cand_in", [128, NCORES, NT3, 8], F32,
                                        isOutput=False)
    sc_in = nc.declare_dram_parameter("sc_in", [128, NT3, 4], F32,
                                      isOutput=False)
    hmrg_in = nc.declare_dram_parameter("hmrg_in", [128, NT3, 2], F32,
                                        isOutput=False)
    out2 = nc.declare_dram_parameter("out2", [2, 1], F32, isOutput=True)

    with tile.TileContext(nc) as tc:
        with (
            tc.tile_pool(name="singles", bufs=1) as singles,
            tc.tile_pool(name="work", bufs=2) as work,
            tc.tile_pool(name="psB", bufs=2, space="PSUM") as psB,
        ):
            cand = singles.tile([128, NCORES, NT3, 8], F32, tag="cand")
            nc.sync.dma_start(out=cand, in_=cand_in[:, :, :, :])
            sc_sb = singles.tile([128, NT3, 4], F32, tag="scsb")
            nc.scalar.dma_start(out=sc_sb, in_=sc_in[:, :, :])
            hmrg_sb = singles.tile([128, NT3, 2], F32, tag="hmrg")
            nc.scalar.dma_start(out=hmrg_sb, in_=hmrg_in[:, :, :])
            onesf_sb = singles.tile([128, 1], F32, tag="onesf")
            nc.vector.memset(onesf_sb, 1.0)

            c1v = hmrg_sb[:, :, 0:1].rearrange("p t o -> p o t") \
                .unsqueeze(-1).to_broadcast([128, NCORES, NT3, 8])
            c0v = sc_sb[:, :, 0:1].rearrange("p t o -> p o t") \
                .unsqueeze(-1).to_broadcast([128, NCORES, NT3, 8])
            d_sb = singles.tile([128, NCORES, NT3, 8], F32, tag="dsb")
            nc.vector.tensor_tensor(out=d_sb, in0=cand, in1=c1v, op=OP.mult)
            nc.vector.tensor_tensor(out=d_sb, in0=d_sb, in1=c0v, op=OP.add)
            t8m = singles.tile([128, NT3, 8], F32, tag="t8m")
            for m in range(NT3):
                nc.vector.max(out=t8m[:, m, :], in_=d_sb[:, :, m, :])
            e6 = work.tile([128, NT3, 6], F32, tag="e6")
            nc.scalar.activation(out=e6, in_=t8m[:, :, 0:6], func=AF.Exp)
            se6 = work.tile([128, NT3], F32, tag="se6")
            nc.vector.reduce_sum(out=se6[:, :].unsqueeze(-1), in_=e6,
                                 axis=mybir.AxisListType.X)
            numer = work.tile([128, NT3], F32, tag="numer")
            nc.scalar.activation(out=numer, in_=sc_sb[:, :, 1], func=AF.Exp)
            mx = work.tile([128, NT3], F32, tag="mx")
            nc.vector.tensor_tensor(out=mx[:, :].unsqueeze(-1),
                                    in0=t8m[:, :, 5:6],
                                    in1=sc_sb[:, :, 1:2], op=OP.max)
            em = work.tile([128, NT3], F32, tag="em")
            nc.scalar.activation(out=em, in_=mx, func=AF.Exp)
            deno = work.tile([128, NT3], F32, tag="deno")
            nc.vector.tensor_tensor(out=deno, in0=se6, in1=em, op=OP.subtract)
            nc.vector.tensor_tensor(out=deno, in0=deno, in1=numer, op=OP.add)
            nc.vector.reciprocal(out=deno, in_=deno)
            nc.vector.tensor_tensor(out=deno, in0=deno, in1=numer, op=OP.mult)
            nc.vector.tensor_tensor(out=deno, in0=deno,
                                    in1=hmrg_sb[:, :, 1], op=OP.mult)
            spm = work.tile([128, NT3], F32, tag="spm")
            nc.vector.tensor_tensor(out=spm, in0=sc_sb[:, :, 2],
                                    in1=hmrg_sb[:, :, 1], op=OP.mult)
            pack2 = work.tile([128, 2], F32, tag="pack2")
            nc.vector.reduce_sum(out=pack2[:, 0:1], in_=deno,
                                 axis=mybir.AxisListType.X)
            nc.vector.reduce_sum(out=pack2[:, 1:2], in_=spm,
                                 axis=mybir.AxisListType.X)
            ps_f = psB.tile([2, 1], F32)
            nc.tensor.matmul(out=ps_f, lhsT=pack2, rhs=onesf_sb,
                             start=True, stop=True)
            fin = work.tile([2, 1], F32, tag="fin")
            nc.vector.tensor_scalar(out=fin, in0=ps_f, scalar1=1.0 / NSEQ,
                                    scalar2=None, op0=OP.mult)
            nc.sync.dma_start(out=out2[:, :], in_=fin)
    nc.compile()
    return nc


_NC_CACHE = {}
LAST_RUNS = []


def _hw_runner(nc, in_maps):
    import os
    res = run_bass_kernel_spmd(
        nc, in_maps, list(range(NCORES)),
        trace=bool(os.environ.get("KERNEL_TRACE")))
    LAST_RUNS.append(res)
    return res.results


def _get(name, builder):
    if name not in _NC_CACHE:
        _NC_CACHE[name] = builder()
    return _NC_CACHE[name]


def kernel(frame_embeds, other_frame_embeds, W, b, bridge, _runner=None):
    frame_embeds = np.asarray(frame_embeds, dtype=np.float32)
    other_frame_embeds = np.asarray(other_frame_embeds, dtype=np.float32)
    W = np.asarray(W, dtype=np.float32)
    b = np.asarray(b, dtype=np.float32)
    bridge = np.asarray(bridge, dtype=np.int32)
    runner = _runner if _runner is not None else _hw_runner

    # ---- host-side sharding / layout (indexing + dtype cast only) ----
    fe_seq = frame_embeds.transpose(0, 2, 1, 3).reshape(NSEQ, T, HID)
    ofe_seq = other_frame_embeds.transpose(0, 2, 1, 3).reshape(NSEQ, T, HID)
    perm = np.argsort(bridge[:, 1], kind="stable")
    fe_sorted = fe_seq[perm]
    bridge_s = bridge[perm].astype(np.float32)

    bh, bp, bt = bridge_s[:, 0], bridge_s[:, 1], bridge_s[:, 2]
    alpha = (bp - bh) / (bt - bh)
    sigma = alpha * (bt - bp)
    c1 = 1.0 / (sigma * sigma)
    piv = bridge[perm][:, 1].astype(np.int64)

    w_host = np.ascontiguousarray(
        W.reshape(2, 128, PROJ).astype(ml_dtypes.bfloat16))
    bc = np.ascontiguousarray(b.reshape(2, 128).T.astype(np.float32))

    in1 = []
    for k in range(NCORES):
        sl = slice(k * SPC, (k + 1) * SPC)
        cur = fe_sorted[sl]                      # (200, 16, 256)
        oth = ofe_seq[sl]                        # (200, 16, 256)
        both_int = np.concatenate([cur[:, 1:T - 1, :], oth[:, 1:T - 1, :]],
                                  axis=0)        # (400, 14, 256)
        xt_int = np.ascontiguousarray(
            both_int.transpose(2, 1, 0).astype(ml_dtypes.bfloat16)) \
            .reshape(2, 128, NG, NCOL)
        g1 = cur[np.arange(SPC), piv[sl], :]     # (200, 256)
        g_cols = np.concatenate([cur[:, 0, :], g1, cur[:, T - 1, :]], axis=0)
        xt_gh = np.ascontiguousarray(
            g_cols.T.astype(ml_dtypes.bfloat16)).reshape(2, 128, GSZ)
        hrow = np.concatenate([
            1.0 - alpha[sl], alpha[sl], -0.5 * c1[sl]]).reshape(1, 3 * SPC)
        in1.append({
            "xt_int": xt_int, "xt_g": xt_gh, "w_in": w_host, "bc_in": bc,
            "hrow_in": np.ascontiguousarray(hrow.astype(np.float32)),
        })

    nc1 = _get("l1", _build_l1)
    r1 = runner(nc1, in1)

    # ---- host: slot layout (pivot groups padded to 32 rows) ----
    counts = np.bincount(piv, minlength=T - 1)[1:T - 1]      # g = 1..14
    caps = ((counts + 31) // 32) * 32
    starts = np.zeros(NG, np.int64)
    starts[1:] = np.cumsum(caps)[:-1]
    nslots = int(caps.sum())
    nt2 = (nslots + 127) // 128
    npad2 = nt2 * 128
    rank = np.arange(NSEQ, dtype=np.int64) - np.concatenate(
        [[0], np.cumsum(counts)])[:-1][piv - 1]
    slot_of = starts[piv - 1] + rank                        # sorted row -> slot

    # block -> pivot group (every 32-block lies in one group's cap region)
    blk_g = np.zeros(npad2 // 32, np.int64)
    for g in range(NG):
        blk_g[starts[g] // 32:(starts[g] + caps[g]) // 32] = g + 1
    blk_g[nslots // 32:] = NG  # tail blocks: any valid group (A cols zero)

    segments = []
    for m in range(nt2):
        blks = blk_g[m * 4:(m + 1) * 4]
        b0 = 0
        while b0 < 4:
            g = int(blks[b0])
            b1 = b0
            while b1 < 4 and blks[b1] == g:
                b1 += 1
            span = b1 - b0
            while span:
                if b0 == 0 and span == 4:
                    sz = 4
                elif b0 % 2 == 0 and span >= 2:
                    sz = 2
                else:
                    sz = 1
                segments.append((m, b0 * 32, (b0 + sz) * 32, g))
                b0 += sz
                span -= sz
    segments = tuple(segments)

    # A [2, 128, npad2] bf16: scatter a-vectors to their slots
    a_all = np.concatenate([r1[k]["a_out"] for k in range(NCORES)], axis=2)
    a_pad = np.zeros((2, 128, npad2), dtype=ml_dtypes.bfloat16)
    a_pad[:, :, slot_of] = a_all
    a_pad = np.ascontiguousarray(a_pad)

    in2 = [{"a_in": a_pad, "pool_in": r1[k]["pool_out"]}
           for k in range(NCORES)]
    key = ("l2", segments, nt2)
    if key not in _NC_CACHE:
        _NC_CACHE[key] = _build_l2(segments, nt2)
    r2 = runner(_NC_CACHE[key], in2)

    # ---- host: gather top8 + scalars into merge layout (indexing) ----
    npad3 = NT3 * 128
    cand = np.zeros((128, NCORES, NT3, 8), np.float32)
    for k in range(NCORES):
        cand[:, k, :nt2, :] = r2[k]["t8_out"].reshape(128, nt2, 8)
    cand = np.ascontiguousarray(cand)
    sc_slots = np.zeros((npad3, 4), np.float32)
    sc_all = np.concatenate(
        [r1[k]["sc_out"].reshape(SPC, 4) for k in range(NCORES)], axis=0)
    sc_slots[slot_of] = sc_all
    sc_in = np.ascontiguousarray(
        sc_slots.reshape(NT3, 128, 4).transpose(1, 0, 2))

    c1_pad = np.zeros(npad3, np.float32)
    c1_pad[slot_of] = c1
    mask_pad = np.zeros(npad3, np.float32)
    mask_pad[slot_of] = 1.0
    hmrg = np.ascontiguousarray(
        np.stack([c1_pad, mask_pad], -1).reshape(NT3, 128, 2)
        .transpose(1, 0, 2))

    in3 = [{"cand_in": cand, "sc_in": sc_in, "hmrg_in": hmrg}
           for _ in range(NCORES)]
    nc3 = _get("l3", _build_l3)
    r3 = runner(nc3, in3)

    out = r3[0]["out2"]
    return (np.asarray(np.float32(out[0, 0])), np.asarray(np.float32(out[1, 0])))


# revision 18
# speedup vs baseline: 1.1242x; 1.1242x over previous
"""Brownian-bridge criterion loss on 8 Trainium2 NeuronCores (3 launches).

Data-parallel over the 1600 pivot-sorted cur sequences (200/core); the
negative pool is column-sharded (each core scores all 1664 row-slots against
its own 400 sequences' pivot frames). Host work between launches is pure
indexing (concat / transpose / gather).

Launch 1 (value-independent): transposed projection ([hid, cols] inputs,
  W stationary) of the 400 seqs' interior frames; column norms via all-ones
  [128,128] stationary matmul (sums broadcast across PSUM partitions) +
  1/sqrt on scalar engine; bias+normalize fused into PSUM evacuation ->
  pool [c, 14, 400] bf16. Small projection of cur head/pivot/tail columns
  -> a-vectors [c, 200], per-row dots via elementwise+ones-matmul, per-row
  scalars (c0, s, softplus) packed [200, 4].

Launch 2 (specialized to the bridge pivot multiset): cross = A^T @ pool_g
  per (row-tile, pivot-run) segment, Max8 per tile -> top8 [128, 13, 8].

Launch 3 (value-independent): merge the 8 cores' top8 (host-gathered into
  [128, 8, 13, 8]), dist = c1*cross + c0, top-8 of 64, exp /
  top-5-excluding-self trick, masked mean -> (brownian, head_tail).
"""

import sys

sys.path.insert(0, "/opt/trn_rl_repo")

import numpy as np
import ml_dtypes

import concourse.bacc as bacc
import concourse.bass as bass
import concourse.mybir as mybir
import concourse.tile as tile
from concourse.bass_utils import run_bass_kernel_spmd

F32 = mybir.dt.float32
FP8 = mybir.dt.float8e4
BF16 = mybir.dt.bfloat16
I32 = mybir.dt.int32
AF = mybir.ActivationFunctionType
OP = mybir.AluOpType

BS, T, Q, HID, PROJ = 16, 16, 100, 256, 256
NSEQ = BS * Q              # 1600 positive sequences
NCORES = 8
SPC = NSEQ // NCORES       # 200 cur sequences per core
NG = T - 2                 # 14 interior pivot positions (1..14)
NCOL = 2 * SPC             # 400 negative-pool columns per core
NT = (NSEQ + 127) // 128   # 13 row tiles
NPAD = NT * 128            # 1664
DELTA = 0.3
GSZ = 3 * SPC              # 600 head/pivot/tail columns
NT3 = 16                   # fixed slot-tile count for launch 3 (2048 slots)


def _build_l1():
    nc = bacc.Bacc("TRN2", target_bir_lowering=False, debug=False,
                   num_devices=NCORES)
    # fp8 inputs, k-halves paired in dim 1 for DoubleRow matmuls.
    # xt carries raw frames; W is pre-scaled by 16 on the host so fp8 stays
    # in normal range (embeddings come out 16x; norms/dots rescale below).
    xt_int = nc.declare_dram_parameter("xt_int", [128, 2, NG, NCOL], FP8,
                                       isOutput=False)
    xt_g = nc.declare_dram_parameter("xt_g", [128, 2, GSZ], FP8,
                                     isOutput=False)
    w_in = nc.declare_dram_parameter("w_in", [128, 2, PROJ], FP8,
                                     isOutput=False)
    bc_in = nc.declare_dram_parameter("bc_in", [128, 2], F32, isOutput=False)
    # [1,200] host scalars: (1-alpha), alpha, -1/(2 sigma^2)
    hrow_in = nc.declare_dram_parameter("hrow_in", [1, 3 * SPC], F32,
                                        isOutput=False)
    # pool/a leave as fp8 scaled 16x (unit-norm embeddings x16)
    pool_out = nc.declare_dram_parameter("pool_out", [128, 2, NG, NCOL], FP8,
                                         isOutput=True)
    a_out = nc.declare_dram_parameter("a_out", [128, 2, SPC], FP8,
                                      isOutput=True)
    sc_out = nc.declare_dram_parameter("sc_out", [1, SPC * 4], F32,
                                       isOutput=True)
    DR = mybir.MatmulPerfMode.DoubleRow

    with tile.TileContext(nc) as tc:
        with (
            tc.tile_pool(name="singles", bufs=1) as singles,
            tc.tile_pool(name="work", bufs=4) as work,
            tc.tile_pool(name="psA", bufs=4, space="PSUM") as psA,
            tc.tile_pool(name="psB", bufs=1, space="PSUM") as psB,
        ):
            w_sb = singles.tile([128, 2, PROJ], FP8, tag="w")
            nc.scalar.dma_start(out=w_sb, in_=w_in[:, :, :])
            bias_sb = singles.tile([128, 2], F32, tag="bias")
            nc.scalar.dma_start(out=bias_sb, in_=bc_in[:, :])
            b16_sb = singles.tile([128, 2], F32, tag="b16")
            nc.vector.tensor_scalar(out=b16_sb, in0=bias_sb, scalar1=16.0,
                                    scalar2=None, op0=OP.mult)
            hrow_sb = singles.tile([1, 3 * SPC], F32, tag="hrow")
            nc.scalar.dma_start(out=hrow_sb, in_=hrow_in[:, :])
            h_oma = hrow_sb[:, 0:SPC]
            h_alp = hrow_sb[:, SPC:2 * SPC]
            h_c1hneg = hrow_sb[:, 2 * SPC:3 * SPC]
            xtg_sb = singles.tile([128, 2, GSZ], FP8, tag="xtg")
            nc.sync.dma_start(out=xtg_sb, in_=xt_g[:, :, :])
            NCHK = 2
            GPC = NG // NCHK  # 7
            xt_sb = [None] * NCHK
            for ck in range(NCHK):
                t_x = singles.tile([128, 2, GPC, NCOL], FP8, tag=f"xt{ck}")
                eng = nc.sync if ck == 0 else nc.gpsimd
                eng.dma_start(out=t_x,
                              in_=xt_int[:, :, ck * GPC:(ck + 1) * GPC, :])
                xt_sb[ck] = t_x

            ones8 = singles.tile([128, 2, 128], FP8, tag="ones8")
            nc.vector.memset(ones8, 1.0)
            ones16row = singles.tile([1, 128], BF16, tag="ones16row")
            nc.vector.memset(ones16row, 16.0)

            pool2 = singles.tile([128, 2, NG, NCOL], FP8, tag="pool2")

            # --- g-cols projection first (only needs xtg + W) ---
            u_g = [work.tile([128, GSZ], F32, tag=f"ug{mh}", name=f"ug{mh}")
                   for mh in range(2)]
            xsq_g = work.tile([128, 2, GSZ], FP8, tag="xsqg")
            for mh in range(2):
                for c2 in range(2):
                    sl = slice(c2 * 300, (c2 + 1) * 300)
                    p = psA.tile([128, 300], F32, tag="p")
                    nc.tensor.matmul(
                        out=p, lhsT=w_sb[:, :, mh * 128:(mh + 1) * 128],
                        rhs=xtg_sb[:, :, sl], start=True, stop=True,
                        perf_mode=DR)
                    # u_g = ps/16 + b  (original embedding scale)
                    nc.vector.tensor_scalar(
                        out=u_g[mh][:, sl], in0=p, scalar1=1.0 / 16.0,
                        scalar2=bias_sb[:, mh:mh + 1], op0=OP.mult,
                        op1=OP.add)
                    nc.scalar.activation(out=xsq_g[:, mh, sl], in_=p,
                                         func=AF.Square, scale=1.0 / 16.0,
                                         bias=bias_sb[:, mh:mh + 1])

            # --- main loop with the g-phase tail interleaved ---
            rinv_g = []
            a_f = [work.tile([128, SPC], F32, tag=f"af{mh}", name=f"af{mh}")
                   for mh in range(2)]
            a_bf = work.tile([128, 2, SPC], FP8, tag="abf")
            pst2 = work.tile([128, 2, 3, SPC], FP8, tag="pst2")
            pd = []
            ps_s = None

            def emit_gn():
                for j in range(3):
                    gn = psB.tile([1, SPC], F32, name=f"gn{j}", tag="acc1",
                                  bufs=1)
                    nc.tensor.matmul(
                        out=gn, lhsT=ones8[:, :, 0:1],
                        rhs=xsq_g[:, :, j * SPC:(j + 1) * SPC],
                        start=True, stop=True, perf_mode=DR)
                    rv = work.tile([1, SPC], F32, tag=f"rinvg{j}",
                                   name=f"rinvg{j}")
                    nc.scalar.activation(out=rv, in_=gn,
                                         func=AF.Abs_reciprocal_sqrt)
                    rinv_g.append(rv)

            def emit_pss():
                nonlocal ps_s
                # s02 = (1-a)/||u0||, a/||u2||; broadcast x16 via the
                # ones16 stationary -> a-vectors come out 16x (fp8-normal)
                s02 = work.tile([1, 2 * SPC], BF16, tag="s02")
                nc.vector.tensor_tensor(out=s02[:, 0:SPC], in0=h_oma,
                                        in1=rinv_g[0], op=OP.mult)
                nc.vector.tensor_tensor(out=s02[:, SPC:2 * SPC], in0=h_alp,
                                        in1=rinv_g[2], op=OP.mult)
                ps_s = psA.tile([128, 2 * SPC], F32, tag="pss", bufs=1)
                nc.tensor.matmul(out=ps_s, lhsT=ones16row, rhs=s02,
                                 start=True, stop=True)

            def emit_abuild():
                for mh in range(2):
                    u0 = u_g[mh][:, 0:SPC]
                    u1 = u_g[mh][:, SPC:2 * SPC]
                    u2 = u_g[mh][:, 2 * SPC:3 * SPC]
                    t1 = work.tile([128, SPC], F32, tag="t1")
                    nc.vector.tensor_tensor(out=t1, in0=u0,
                                            in1=ps_s[:, 0:SPC], op=OP.mult)
                    t2 = work.tile([128, SPC], F32, tag="t2")
                    nc.vector.tensor_tensor(out=t2, in0=u2,
                                            in1=ps_s[:, SPC:2 * SPC],
                                            op=OP.mult)
                    nc.vector.tensor_tensor(out=a_f[mh], in0=t1, in1=t2,
                                            op=OP.add)
                    nc.vector.tensor_copy(out=a_bf[:, mh, :], in_=a_f[mh])
                    nc.vector.tensor_tensor(out=pst2[:, mh, 0, :],
                                            in0=a_f[mh], in1=u1, op=OP.mult)
                    nc.vector.tensor_tensor(out=pst2[:, mh, 1, :],
                                            in0=a_f[mh], in1=a_f[mh],
                                            op=OP.mult)
                    nc.vector.tensor_tensor(out=pst2[:, mh, 2, :], in0=u0,
                                            in1=u2, op=OP.mult)
                nc.gpsimd.dma_start(out=a_out[:, :, :], in_=a_bf)

            def emit_pd():
                for j in range(3):
                    pdj = psB.tile([1, SPC], F32, name=f"pd{j}", tag="acc1",
                                   bufs=1)
                    nc.tensor.matmul(
                        out=pdj, lhsT=ones8[:, :, 0:1],
                        rhs=pst2[:, :, j, :], start=True, stop=True,
                        perf_mode=DR)
                    pd.append(pdj)

            def emit_scmath():
                # a' = 16a: pd0 = 16(a.u1), pd1 = 256(a.a), pd2 = u0.u2
                sc_pack = work.tile([1, SPC, 4], F32, tag="scpack")
                nc.vector.memset(sc_pack[:, :, 3:4], 0.0)
                q_r = work.tile([1, SPC], F32, tag="qr")
                nc.vector.tensor_tensor(out=q_r, in0=pd[0], in1=rinv_g[1],
                                        op=OP.mult)
                aa_r = work.tile([1, SPC], F32, tag="aar")
                nc.vector.tensor_scalar(out=aa_r, in0=pd[1],
                                        scalar1=1.0 / 256.0, scalar2=None,
                                        op0=OP.mult)
                sc_r = work.tile([1, SPC], F32, tag="scr")
                nc.vector.tensor_tensor(out=sc_r, in0=pd[2], in1=rinv_g[0],
                                        op=OP.mult)
                nc.vector.tensor_tensor(out=sc_r, in0=sc_r, in1=rinv_g[2],
                                        op=OP.mult)
                # c0 = (1 + aa) * (-1/(2 sigma^2))
                nc.vector.scalar_tensor_tensor(
                    out=sc_pack[:, :, 0], in0=aa_r, scalar=1.0,
                    in1=h_c1hneg, op0=OP.add, op1=OP.mult)
                # s = (1 - 2q + aa) * (-1/(2 sigma^2)); q = q_r/16
                t_r = work.tile([1, SPC], F32, tag="tr")
                nc.vector.scalar_tensor_tensor(
                    out=t_r, in0=q_r, scalar=-0.125, in1=aa_r,
                    op0=OP.mult, op1=OP.add)
                nc.vector.tensor_scalar(out=t_r, in0=t_r, scalar1=1.0,
                                        scalar2=None, op0=OP.add)
                nc.vector.tensor_tensor(out=sc_pack[:, :, 1], in0=t_r,
                                        in1=h_c1hneg, op=OP.mult)
                # softplus(delta - score)
                delta_sb = work.tile([1, 1], F32, tag="delta")
                nc.vector.memset(delta_sb, DELTA)
                e_r = work.tile([1, SPC], F32, tag="er")
                nc.scalar.activation(out=e_r, in_=sc_r, func=AF.Exp,
                                     scale=-1.0, bias=delta_sb)
                nc.scalar.activation(out=sc_pack[:, :, 2], in_=e_r,
                                     func=AF.Ln, bias=1.0)
                nc.gpsimd.dma_start(
                    out=sc_out[:, :],
                    in_=sc_pack[:, :, :].rearrange("o s q -> o (s q)"))

            inject = {1: emit_gn, 2: emit_pss, 3: emit_abuild, 5: emit_pd,
                      6: emit_scmath}
            for gp in range(NG // 2):
                nbp = psB.tile([128, 1024], F32, tag="accb", bufs=1)
                for g2 in range(2):
                    g = 2 * gp + g2
                    ck, go = g // GPC, g % GPC
                    ps = [None, None]
                    for mh in range(2):
                        p = psA.tile([128, NCOL], F32, tag="p",
                                     name=f"p{g2}{mh}")
                        nc.tensor.matmul(
                            out=p, lhsT=w_sb[:, :, mh * 128:(mh + 1) * 128],
                            rhs=xt_sb[ck][:, :, go, :], start=True,
                            stop=True, perf_mode=DR)
                        ps[mh] = p
                    xsq2 = work.tile([128, 2, NCOL], FP8, tag="xsq2")
                    for mh in range(2):
                        nc.scalar.activation(out=xsq2[:, mh, :], in_=ps[mh],
                                             func=AF.Square, scale=1.0 / 16.0,
                                             bias=bias_sb[:, mh:mh + 1])
                    nc.tensor.matmul(out=nbp[:, g2 * 512:g2 * 512 + NCOL],
                                     lhsT=ones8, rhs=xsq2, start=True,
                                     stop=True, perf_mode=DR)
                    if g2 == 0:
                        ps_pair = [ps]
                    else:
                        ps_pair.append(ps)
                rinvb = work.tile([128, 2, NCOL], BF16, tag="rinvb")
                nc.scalar.activation(
                    out=rinvb,
                    in_=nbp[:, :].rearrange("x (b n) -> x b n", b=2)[:, :,
                                                                    0:NCOL],
                    func=AF.Abs_reciprocal_sqrt)
                for g2 in range(2):
                    g = 2 * gp + g2
                    for mh in range(2):
                        nc.vector.scalar_tensor_tensor(
                            out=pool2[:, mh, g, :], in0=ps_pair[g2][mh],
                            scalar=b16_sb[:, mh:mh + 1],
                            in1=rinvb[:, g2, :], op0=OP.add, op1=OP.mult)
                if gp in inject:
                    inject[gp]()
                if gp == 3:
                    nc.sync.dma_start(out=pool_out[:, :, 0:8, :],
                                      in_=pool2[:, :, 0:8, :])
            nc.sync.dma_start(out=pool_out[:, :, 8:NG, :],
                              in_=pool2[:, :, 8:NG, :])
    nc.compile()
    return nc


def _build_l2(segments, nt2):
    """segments: tuple of (tile m, p0, p1, pivot g); p0/p1 32-aligned so
    every sub-matmul lands on a valid PE tile position."""
    nc = bacc.Bacc("TRN2", target_bir_lowering=False, debug=False,
                   num_devices=NCORES)
    a_in = nc.declare_dram_parameter("a_in", [128, 2, nt2 * 128], FP8,
                                     isOutput=False)
    pool_in = nc.declare_dram_parameter("pool_in", [128, 2, NG, NCOL], FP8,
                                        isOutput=False)
    t8_out = nc.declare_dram_parameter("t8_out", [128, nt2 * 8], F32,
                                       isOutput=True)
    DR = mybir.MatmulPerfMode.DoubleRow

    with tile.TileContext(nc) as tc:
        with (
            tc.tile_pool(name="singles", bufs=1) as singles,
            tc.tile_pool(name="psA", bufs=6, space="PSUM") as psA,
        ):
            A_sb = singles.tile([128, 2, nt2 * 128], FP8, tag="A")
            nc.sync.dma_start(out=A_sb, in_=a_in[:, :, :])
            pool_sb = singles.tile([128, 2, NG, NCOL], FP8, tag="pool")
            for ck in range(2):
                gsl = slice(ck * 7, (ck + 1) * 7)
                eng = nc.scalar if ck == 0 else nc.gpsimd
                eng.dma_start(out=pool_sb[:, :, gsl, :],
                              in_=pool_in[:, :, gsl, :])

            t8_sb = singles.tile([128, nt2, 8], F32, tag="t8")
            for m in range(nt2):
                px = psA.tile([128, NCOL], F32, tag="px")
                for (sm, p0, p1, g) in segments:
                    if sm != m:
                        continue
                    if p0 == 0 and p1 == 128:
                        nc.tensor.matmul(
                            out=px[p0:p1, :],
                            lhsT=A_sb[:, :, m * 128 + p0:m * 128 + p1],
                            rhs=pool_sb[:, :, g - 1, :],
                            start=True, stop=True, perf_mode=DR)
                    else:
                        for kh in range(2):
                            nc.tensor.matmul(
                                out=px[p0:p1, :],
                                lhsT=A_sb[:, kh, m * 128 + p0:m * 128 + p1],
                                rhs=pool_sb[:, kh, g - 1, :],
                                start=(kh == 0), stop=(kh == 1))
                nc.vector.max(out=t8_sb[:, m, :], in_=px)
            nc.gpsimd.dma_start(
                out=t8_out[:, :],
                in_=t8_sb[:, :, :].rearrange("p t e -> p (t e)"))
    nc.compile()
    return nc


def _build_l3():
    nc = bacc.Bacc("TRN2", target_bir_lowering=False, debug=False,
                   num_devices=NCORES)
    cand_in = nc.declare_dram_parameter("cand_in", [128, NCORES, NT3, 8], F32,
                                        isOutput=False)
    sc_in = nc.declare_dram_parameter("sc_in", [128, NT3, 4], F32,
                                      isOutput=False)
    hmrg_in = nc.declare_dram_parameter("hmrg_in", [128, NT3, 2], F32,
                                        isOutput=False)
    out2 = nc.declare_dram_parameter("out2", [2, 1], F32, isOutput=True)

    with tile.TileContext(nc) as tc:
        with (
            tc.tile_pool(name="singles", bufs=1) as singles,
            tc.tile_pool(name="work", bufs=2) as work,
            tc.tile_pool(name="psB", bufs=2, space="PSUM") as psB,
        ):
            cand = singles.tile([128, NCORES, NT3, 8], F32, tag="cand")
            nc.sync.dma_start(out=cand, in_=cand_in[:, :, :, :])
            sc_sb = singles.tile([128, NT3, 4], F32, tag="scsb")
            nc.scalar.dma_start(out=sc_sb, in_=sc_in[:, :, :])
            hmrg_sb = singles.tile([128, NT3, 2], F32, tag="hmrg")
            nc.scalar.dma_start(out=hmrg_sb, in_=hmrg_in[:, :, :])
            onesf_sb = singles.tile([128, 1], F32, tag="onesf")
            nc.vector.memset(onesf_sb, 1.0)

            c1v = hmrg_sb[:, :, 0:1].rearrange("p t o -> p o t") \
                .unsqueeze(-1).to_broadcast([128, NCORES, NT3, 8])
            c0v = sc_sb[:, :, 0:1].rearrange("p t o -> p o t") \
                .unsqueeze(-1).to_broadcast([128, NCORES, NT3, 8])
            d_sb = singles.tile([128, NCORES, NT3, 8], F32, tag="dsb")
            nc.vector.tensor_tensor(out=d_sb, in0=cand, in1=c1v, op=OP.mult)
            nc.vector.tensor_tensor(out=d_sb, in0=d_sb, in1=c0v, op=OP.add)
            t8m = singles.tile([128, NT3, 8], F32, tag="t8m")
            for m in range(NT3):
                nc.vector.max(out=t8m[:, m, :], in_=d_sb[:, :, m, :])
            e6 = work.tile([128, NT3, 6], F32, tag="e6")
            nc.scalar.activation(out=e6, in_=t8m[:, :, 0:6], func=AF.Exp)
            se6 = work.tile([128, NT3], F32, tag="se6")
            nc.vector.reduce_sum(out=se6[:, :].unsqueeze(-1), in_=e6,
                                 axis=mybir.AxisListType.X)
            numer = work.tile([128, NT3], F32, tag="numer")
            nc.scalar.activation(out=numer, in_=sc_sb[:, :, 1], func=AF.Exp)
            mx = work.tile([128, NT3], F32, tag="mx")
            nc.vector.tensor_tensor(out=mx[:, :].unsqueeze(-1),
                                    in0=t8m[:, :, 5:6],
                                    in1=sc_sb[:, :, 1:2], op=OP.max)
            em = work.tile([128, NT3], F32, tag="em")
            nc.scalar.activation(out=em, in_=mx, func=AF.Exp)
            deno = work.tile([128, NT3], F32, tag="deno")
            nc.vector.tensor_tensor(out=deno, in0=se6, in1=em, op=OP.subtract)
            nc.vector.tensor_tensor(out=deno, in0=deno, in1=numer, op=OP.add)
            nc.vector.reciprocal(out=deno, in_=deno)
            nc.vector.tensor_tensor(out=deno, in0=deno, in1=numer, op=OP.mult)
            nc.vector.tensor_tensor(out=deno, in0=deno,
                                    in1=hmrg_sb[:, :, 1], op=OP.mult)
            spm = work.tile([128, NT3], F32, tag="spm")
            nc.vector.tensor_tensor(out=spm, in0=sc_sb[:, :, 2],
                                    in1=hmrg_sb[:, :, 1], op=OP.mult)
            pack2 = work.tile([128, 2], F32, tag="pack2")
            nc.vector.reduce_sum(out=pack2[:, 0:1], in_=deno,
                                 axis=mybir.AxisListType.X)
            nc.vector.reduce_sum(out=pack2[:, 1:2], in_=spm,
                                 axis=mybir.AxisListType.X)
            ps_f = psB.tile([2, 1], F32)
            nc.tensor.matmul(out=ps_f, lhsT=pack2, rhs=onesf_sb,
                             start=True, stop=True)
            fin = work.tile([2, 1], F32, tag="fin")
            nc.vector.tensor_scalar(out=fin, in0=ps_f, scalar1=1.0 / NSEQ,
                                    scalar2=None, op0=OP.mult)
            nc.sync.dma_start(out=out2[:, :], in_=fin)
    nc.compile()
    return nc


_NC_CACHE = {}
LAST_RUNS = []


def _hw_runner(nc, in_maps):
    import os
    res = run_bass_kernel_spmd(
        nc, in_maps, list(range(NCORES)),
        trace=bool(os.environ.get("KERNEL_TRACE")))
    LAST_RUNS.append(res)
    return res.results


def _get(name, builder):
    if name not in _NC_CACHE:
        _NC_CACHE[name] = builder()
    return _NC_CACHE[name]


def kernel(frame_embeds, other_frame_embeds, W, b, bridge, _runner=None):
    frame_embeds = np.asarray(frame_embeds, dtype=np.float32)
    other_frame_embeds = np.asarray(other_frame_embeds, dtype=np.float32)
    W = np.asarray(W, dtype=np.float32)
    b = np.asarray(b, dtype=np.float32)
    bridge = np.asarray(bridge, dtype=np.int32)
    runner = _runner if _runner is not None else _hw_runner

    # ---- host-side sharding / layout (indexing + dtype cast only) ----
    fe_seq = frame_embeds.transpose(0, 2, 1, 3).reshape(NSEQ, T, HID)
    ofe_seq = other_frame_embeds.transpose(0, 2, 1, 3).reshape(NSEQ, T, HID)
    perm = np.argsort(bridge[:, 1], kind="stable")
    fe_sorted = fe_seq[perm]
    bridge_s = bridge[perm].astype(np.float32)

    bh, bp, bt = bridge_s[:, 0], bridge_s[:, 1], bridge_s[:, 2]
    alpha = (bp - bh) / (bt - bh)
    sigma = alpha * (bt - bp)
    c1 = 1.0 / (sigma * sigma)
    piv = bridge[perm][:, 1].astype(np.int64)

    NP8 = mybir.dt.np(FP8)
    # W scaled x16 so fp8 entries sit in normal range; [p, kh, c] layout
    w_host = np.ascontiguousarray(
        (16.0 * W).reshape(2, 128, PROJ).transpose(1, 0, 2).astype(NP8))
    bc = np.ascontiguousarray(b.reshape(2, 128).T.astype(np.float32))

    in1 = []
    for k in range(NCORES):
        sl = slice(k * SPC, (k + 1) * SPC)
        cur = fe_sorted[sl]                      # (200, 16, 256)
        oth = ofe_seq[sl]                        # (200, 16, 256)
        both_int = np.concatenate([cur[:, 1:T - 1, :], oth[:, 1:T - 1, :]],
                                  axis=0)        # (400, 14, 256)
        # [hid, g, col] -> [p, kh, g, col]
        xt_int = np.ascontiguousarray(
            both_int.transpose(2, 1, 0).reshape(2, 128, NG, NCOL)
            .transpose(1, 0, 2, 3).astype(NP8))
        g1 = cur[np.arange(SPC), piv[sl], :]     # (200, 256)
        g_cols = np.concatenate([cur[:, 0, :], g1, cur[:, T - 1, :]], axis=0)
        xt_gh = np.ascontiguousarray(
            g_cols.T.reshape(2, 128, GSZ).transpose(1, 0, 2).astype(NP8))
        hrow = np.concatenate([
            1.0 - alpha[sl], alpha[sl], -0.5 * c1[sl]]).reshape(1, 3 * SPC)
        in1.append({
            "xt_int": xt_int, "xt_g": xt_gh, "w_in": w_host, "bc_in": bc,
            "hrow_in": np.ascontiguousarray(hrow.astype(np.float32)),
        })

    nc1 = _get("l1", _build_l1)
    r1 = runner(nc1, in1)

    # ---- host: slot layout (pivot groups padded to 32 rows) ----
    counts = np.bincount(piv, minlength=T - 1)[1:T - 1]      # g = 1..14
    caps = ((counts + 31) // 32) * 32
    starts = np.zeros(NG, np.int64)
    starts[1:] = np.cumsum(caps)[:-1]
    nslots = int(caps.sum())
    nt2 = (nslots + 127) // 128
    npad2 = nt2 * 128
    rank = np.arange(NSEQ, dtype=np.int64) - np.concatenate(
        [[0], np.cumsum(counts)])[:-1][piv - 1]
    slot_of = starts[piv - 1] + rank                        # sorted row -> slot

    # block -> pivot group (every 32-block lies in one group's cap region)
    blk_g = np.zeros(npad2 // 32, np.int64)
    for g in range(NG):
        blk_g[starts[g] // 32:(starts[g] + caps[g]) // 32] = g + 1
    blk_g[nslots // 32:] = NG  # tail blocks: any valid group (A cols zero)

    segments = []
    for m in range(nt2):
        blks = blk_g[m * 4:(m + 1) * 4]
        b0 = 0
        while b0 < 4:
            g = int(blks[b0])
            b1 = b0
            while b1 < 4 and blks[b1] == g:
                b1 += 1
            span = b1 - b0
            while span:
                if b0 == 0 and span == 4:
                    sz = 4
                elif b0 % 2 == 0 and span >= 2:
                    sz = 2
                else:
                    sz = 1
                segments.append((m, b0 * 32, (b0 + sz) * 32, g))
                b0 += sz
                span -= sz
    segments = tuple(segments)

    # A [128, 2, npad2] fp8: scatter a-vectors to their slots
    a_all = np.concatenate([r1[k]["a_out"] for k in range(NCORES)], axis=2)
    a_pad = np.zeros((128, 2, npad2), dtype=NP8)
    a_pad[:, :, slot_of] = a_all
    a_pad = np.ascontiguousarray(a_pad)

    in2 = [{"a_in": a_pad, "pool_in": r1[k]["pool_out"]}
           for k in range(NCORES)]
    key = ("l2", segments, nt2)
    if key not in _NC_CACHE:
        _NC_CACHE[key] = _build_l2(segments, nt2)
    r2 = runner(_NC_CACHE[key], in2)

    # ---- host: gather top8 + scalars into merge layout (indexing) ----
    npad3 = NT3 * 128
    cand = np.zeros((128, NCORES, NT3, 8), np.float32)
    for k in range(NCORES):
        cand[:, k, :nt2, :] = r2[k]["t8_out"].reshape(128, nt2, 8)
    cand = np.ascontiguousarray(cand)
    sc_slots = np.zeros((npad3, 4), np.float32)
    sc_all = np.concatenate(
        [r1[k]["sc_out"].reshape(SPC, 4) for k in range(NCORES)], axis=0)
    sc_slots[slot_of] = sc_all
    sc_in = np.ascontiguousarray(
        sc_slots.reshape(NT3, 128, 4).transpose(1, 0, 2))

    c1_pad = np.zeros(npad3, np.float32)
    c1_pad[slot_of] = c1 / 256.0
    mask_pad = np.zeros(npad3, np.float32)
    mask_pad[slot_of] = 1.0
    hmrg = np.ascontiguousarray(
        np.stack([c1_pad, mask_pad], -1).reshape(NT3, 128, 2)
        .transpose(1, 0, 2))

    in3 = [{"cand_in": cand, "sc_in": sc_in, "hmrg_in": hmrg}
           for _ in range(NCORES)]
    nc3 = _get("l3", _build_l3)
    r3 = runner(nc3, in3)

    out = r3[0]["out2"]
    return (np.asarray(np.float32(out[0, 0])), np.asarray(np.float32(out[1, 0])))


# revision 24
# speedup vs baseline: 1.1262x; 1.0018x over previous
"""Brownian-bridge criterion loss on 8 Trainium2 NeuronCores (3 launches).

Data-parallel over the 1600 pivot-sorted cur sequences (200/core); the
negative pool is column-sharded (each core scores all 1664 row-slots against
its own 400 sequences' pivot frames). Host work between launches is pure
indexing (concat / transpose / gather).

Launch 1 (value-independent): transposed projection ([hid, cols] inputs,
  W stationary) of the 400 seqs' interior frames; column norms via all-ones
  [128,128] stationary matmul (sums broadcast across PSUM partitions) +
  1/sqrt on scalar engine; bias+normalize fused into PSUM evacuation ->
  pool [c, 14, 400] bf16. Small projection of cur head/pivot/tail columns
  -> a-vectors [c, 200], per-row dots via elementwise+ones-matmul, per-row
  scalars (c0, s, softplus) packed [200, 4].

Launch 2 (specialized to the bridge pivot multiset): cross = A^T @ pool_g
  per (row-tile, pivot-run) segment, Max8 per tile -> top8 [128, 13, 8].

Launch 3 (value-independent): merge the 8 cores' top8 (host-gathered into
  [128, 8, 13, 8]), dist = c1*cross + c0, top-8 of 64, exp /
  top-5-excluding-self trick, masked mean -> (brownian, head_tail).
"""

import sys

sys.path.insert(0, "/opt/trn_rl_repo")

import numpy as np
import ml_dtypes

import concourse.bacc as bacc
import concourse.bass as bass
import concourse.mybir as mybir
import concourse.tile as tile
from concourse.bass_utils import run_bass_kernel_spmd

F32 = mybir.dt.float32
FP8 = mybir.dt.float8e4
BF16 = mybir.dt.bfloat16
I32 = mybir.dt.int32
AF = mybir.ActivationFunctionType
OP = mybir.AluOpType

BS, T, Q, HID, PROJ = 16, 16, 100, 256, 256
NSEQ = BS * Q              # 1600 positive sequences
NCORES = 8
SPC = NSEQ // NCORES       # 200 cur sequences per core
NG = T - 2                 # 14 interior pivot positions (1..14)
NCOL = 2 * SPC             # 400 negative-pool columns per core
NT = (NSEQ + 127) // 128   # 13 row tiles
NPAD = NT * 128            # 1664
DELTA = 0.3
GSZ = 3 * SPC              # 600 head/pivot/tail columns
NT3 = 16                   # fixed slot-tile count for launch 3 (2048 slots)


def _build_l1():
    nc = bacc.Bacc("TRN2", target_bir_lowering=False, debug=False,
                   num_devices=NCORES)
    # fp8 inputs, k-halves paired in dim 1 for DoubleRow matmuls.
    # xt carries raw frames; W is pre-scaled by 16 on the host so fp8 stays
    # in normal range (embeddings come out 16x; norms/dots rescale below).
    xt_int = nc.declare_dram_parameter("xt_int", [128, 2, NG, NCOL], FP8,
                                       isOutput=False)
    xt_g = nc.declare_dram_parameter("xt_g", [128, 2, GSZ], FP8,
                                     isOutput=False)
    w_in = nc.declare_dram_parameter("w_in", [128, 2, PROJ], FP8,
                                     isOutput=False)
    # bc cols: 0,1 = b (mh halves); 2,3 = 16b; 4 = ||b||^2 (replicated)
    bc_in = nc.declare_dram_parameter("bc_in", [128, 5], F32, isOutput=False)
    wb_in = nc.declare_dram_parameter("wb_in", [128, 2, 128], FP8,
                                      isOutput=False)
    # [1,200] host scalars: (1-alpha), alpha, -1/(2 sigma^2)
    hrow_in = nc.declare_dram_parameter("hrow_in", [1, 3 * SPC], F32,
                                        isOutput=False)
    # pool/a leave as fp8 scaled 16x (unit-norm embeddings x16)
    pool_out = nc.declare_dram_parameter("pool_out", [128, 2, NG, NCOL], FP8,
                                         isOutput=True)
    a_out = nc.declare_dram_parameter("a_out", [128, 2, SPC], FP8,
                                      isOutput=True)
    sc_out = nc.declare_dram_parameter("sc_out", [1, SPC * 4], F32,
                                       isOutput=True)
    DR = mybir.MatmulPerfMode.DoubleRow

    with tile.TileContext(nc) as tc:
        with (
            tc.tile_pool(name="singles", bufs=1) as singles,
            tc.tile_pool(name="work", bufs=4) as work,
            tc.tile_pool(name="psA", bufs=4, space="PSUM") as psA,
            tc.tile_pool(name="psB", bufs=1, space="PSUM") as psB,
        ):
            w_sb = singles.tile([128, 2, PROJ], FP8, tag="w")
            nc.sync.dma_start(out=w_sb, in_=w_in[:, :, :])
            bc_sb = singles.tile([128, 5], F32, tag="bc")
            nc.gpsimd.dma_start(out=bc_sb, in_=bc_in[:, :])
            bias_sb = bc_sb[:, 0:2]
            bb_sb = bc_sb[:, 4:5]
            wb_sb = singles.tile([128, 2, 128], FP8, tag="wb")
            nc.gpsimd.dma_start(out=wb_sb, in_=wb_in[:, :, :])
            hrow_sb = singles.tile([1, 3 * SPC], F32, tag="hrow")
            nc.gpsimd.dma_start(out=hrow_sb, in_=hrow_in[:, :])
            h_oma = hrow_sb[:, 0:SPC]
            h_alp = hrow_sb[:, SPC:2 * SPC]
            h_c1hneg = hrow_sb[:, 2 * SPC:3 * SPC]
            xtg_sb = singles.tile([128, 2, GSZ], FP8, tag="xtg")
            nc.sync.dma_start(out=xtg_sb, in_=xt_g[:, :, :])
            NCHK = 2
            GPC = NG // NCHK  # 7
            xt_sb = [None] * NCHK
            for ck in range(NCHK):
                t_x = singles.tile([128, 2, GPC, NCOL], FP8, tag=f"xt{ck}")
                eng = nc.sync if ck == 0 else nc.gpsimd
                eng.dma_start(out=t_x,
                              in_=xt_int[:, :, ck * GPC:(ck + 1) * GPC, :])
                xt_sb[ck] = t_x

            ones8 = singles.tile([128, 2, 128], FP8, tag="ones8")
            nc.vector.memset(ones8, 1.0)
            ones16row = singles.tile([1, 128], BF16, tag="ones16row")
            nc.vector.memset(ones16row, 16.0)

            pool2 = singles.tile([128, 2, NG, NCOL], FP8, tag="pool2")

            # --- g-cols projection first (only needs xtg + W) ---
            u_g = [work.tile([128, GSZ], F32, tag=f"ug{mh}", name=f"ug{mh}")
                   for mh in range(2)]
            xsq_g = work.tile([128, 2, GSZ], FP8, tag="xsqg")
            for mh in range(2):
                for c2 in range(2):
                    sl = slice(c2 * 300, (c2 + 1) * 300)
                    p = psA.tile([128, 300], F32, tag="pg", bufs=2,
                                 name="pgg", padded_shape=[128, 1024])
                    nc.tensor.matmul(
                        out=p, lhsT=w_sb[:, :, mh * 128:(mh + 1) * 128],
                        rhs=xtg_sb[:, :, sl], start=True, stop=True,
                        perf_mode=DR)
                    # u_g = ps/16 + b  (original embedding scale)
                    nc.vector.tensor_scalar(
                        out=u_g[mh][:, sl], in0=p, scalar1=1.0 / 16.0,
                        scalar2=bias_sb[:, mh:mh + 1], op0=OP.mult,
                        op1=OP.add)
                    # xsq = 4u^2 (bias-free; wb matmul supplies the 2u.b term)
                    nc.scalar.activation(out=xsq_g[:, mh, sl], in_=p,
                                         func=AF.Square, scale=1.0 / 8.0)

            # --- main loop with the g-phase tail interleaved ---
            rinv_g = []
            a_f = [work.tile([128, SPC], F32, tag=f"af{mh}", name=f"af{mh}")
                   for mh in range(2)]
            a_bf = work.tile([128, 2, SPC], FP8, tag="abf")
            pst2 = work.tile([128, 2, 3, SPC], FP8, tag="pst2")
            pd = []
            ps_s = None

            def emit_gn():
                for j in range(3):
                    gn = psB.tile([1, SPC], F32, name=f"gn{j}", tag="acc1",
                                  bufs=1)
                    nc.tensor.matmul(
                        out=gn, lhsT=ones8[:, :, 0:1],
                        rhs=xsq_g[:, :, j * SPC:(j + 1) * SPC],
                        start=True, stop=False, perf_mode=DR)
                    nc.tensor.matmul(
                        out=gn, lhsT=wb_sb[:, :, 0:1],
                        rhs=xtg_sb[:, :, j * SPC:(j + 1) * SPC],
                        start=False, stop=True, perf_mode=DR)
                    rv = work.tile([1, SPC], F32, tag=f"rinvg{j}",
                                   name=f"rinvg{j}")
                    nc.scalar.activation(out=rv, in_=gn, scale=0.25,
                                         bias=bb_sb[0:1, :],
                                         func=AF.Abs_reciprocal_sqrt)
                    rinv_g.append(rv)

            def emit_pss():
                nonlocal ps_s
                # s02 = (1-a)/||u0||, a/||u2||; broadcast x16 via the
                # ones16 stationary -> a-vectors come out 16x (fp8-normal)
                s02 = work.tile([1, 2 * SPC], BF16, tag="s02")
                nc.vector.tensor_tensor(out=s02[:, 0:SPC], in0=h_oma,
                                        in1=rinv_g[0], op=OP.mult)
                nc.vector.tensor_tensor(out=s02[:, SPC:2 * SPC], in0=h_alp,
                                        in1=rinv_g[2], op=OP.mult)
                ps_s = psA.tile([128, 2 * SPC], F32, tag="pss", bufs=1)
                nc.tensor.matmul(out=ps_s, lhsT=ones16row, rhs=s02,
                                 start=True, stop=True)

            def emit_abuild():
                for mh in range(2):
                    u0 = u_g[mh][:, 0:SPC]
                    u1 = u_g[mh][:, SPC:2 * SPC]
                    u2 = u_g[mh][:, 2 * SPC:3 * SPC]
                    t1 = work.tile([128, SPC], F32, tag="t1")
                    nc.vector.tensor_tensor(out=t1, in0=u0,
                                            in1=ps_s[:, 0:SPC], op=OP.mult)
                    t2 = work.tile([128, SPC], F32, tag="t2")
                    nc.vector.tensor_tensor(out=t2, in0=u2,
                                            in1=ps_s[:, SPC:2 * SPC],
                                            op=OP.mult)
                    nc.vector.tensor_tensor(out=a_f[mh], in0=t1, in1=t2,
                                            op=OP.add)
                    nc.vector.tensor_copy(out=a_bf[:, mh, :], in_=a_f[mh])
                    nc.vector.tensor_tensor(out=pst2[:, mh, 0, :],
                                            in0=a_f[mh], in1=u1, op=OP.mult)
                    nc.vector.tensor_tensor(out=pst2[:, mh, 1, :],
                                            in0=a_f[mh], in1=a_f[mh],
                                            op=OP.mult)
                    nc.vector.tensor_tensor(out=pst2[:, mh, 2, :], in0=u0,
                                            in1=u2, op=OP.mult)
                nc.gpsimd.dma_start(out=a_out[:, :, :], in_=a_bf)

            def emit_pd():
                for j in range(3):
                    pdj = psB.tile([1, SPC], F32, name=f"pd{j}", tag="acc1",
                                   bufs=1)
                    nc.tensor.matmul(
                        out=pdj, lhsT=ones8[:, :, 0:1],
                        rhs=pst2[:, :, j, :], start=True, stop=True,
                        perf_mode=DR)
                    pd.append(pdj)

            def emit_scmath():
                # a' = 16a: pd0 = 16(a.u1), pd1 = 256(a.a), pd2 = u0.u2
                sc_pack = work.tile([1, SPC, 4], F32, tag="scpack")
                nc.vector.memset(sc_pack[:, :, 3:4], 0.0)
                q_r = work.tile([1, SPC], F32, tag="qr")
                nc.vector.tensor_tensor(out=q_r, in0=pd[0], in1=rinv_g[1],
                                        op=OP.mult)
                aa_r = work.tile([1, SPC], F32, tag="aar")
                nc.vector.tensor_scalar(out=aa_r, in0=pd[1],
                                        scalar1=1.0 / 256.0, scalar2=None,
                                        op0=OP.mult)
                sc_r = work.tile([1, SPC], F32, tag="scr")
                nc.vector.tensor_tensor(out=sc_r, in0=pd[2], in1=rinv_g[0],
                                        op=OP.mult)
                nc.vector.tensor_tensor(out=sc_r, in0=sc_r, in1=rinv_g[2],
                                        op=OP.mult)
                # c0 = (1 + aa) * (-1/(2 sigma^2))
                nc.vector.scalar_tensor_tensor(
                    out=sc_pack[:, :, 0], in0=aa_r, scalar=1.0,
                    in1=h_c1hneg, op0=OP.add, op1=OP.mult)
                # s = (1 - 2q + aa) * (-1/(2 sigma^2)); q = q_r/16
                t_r = work.tile([1, SPC], F32, tag="tr")
                nc.vector.scalar_tensor_tensor(
                    out=t_r, in0=q_r, scalar=-0.125, in1=aa_r,
                    op0=OP.mult, op1=OP.add)
                nc.vector.tensor_scalar(out=t_r, in0=t_r, scalar1=1.0,
                                        scalar2=None, op0=OP.add)
                nc.vector.tensor_tensor(out=sc_pack[:, :, 1], in0=t_r,
                                        in1=h_c1hneg, op=OP.mult)
                # softplus(delta - score)
                delta_sb = work.tile([1, 1], F32, tag="delta")
                nc.vector.memset(delta_sb, DELTA)
                e_r = work.tile([1, SPC], F32, tag="er")
                nc.scalar.activation(out=e_r, in_=sc_r, func=AF.Exp,
                                     scale=-1.0, bias=delta_sb)
                nc.scalar.activation(out=sc_pack[:, :, 2], in_=e_r,
                                     func=AF.Ln, bias=1.0)
                nc.gpsimd.dma_start(
                    out=sc_out[:, :],
                    in_=sc_pack[:, :, :].rearrange("o s q -> o (s q)"))

            inject = {1: emit_gn, 2: emit_pss, 3: emit_abuild, 5: emit_pd,
                      6: emit_scmath}
            for gp in range(NG // 2):
                nbp = psB.tile([128, 1024], F32, tag="accb", bufs=1)
                pgs = []
                for g2 in range(2):
                    g = 2 * gp + g2
                    ck, go = g // GPC, g % GPC
                    pg = psA.tile([128, 1024], F32, tag="pg",
                                  name=f"pg{g2}", bufs=2)
                    for mh in range(2):
                        nc.tensor.matmul(
                            out=pg[:, mh * 512:mh * 512 + NCOL],
                            lhsT=w_sb[:, :, mh * 128:(mh + 1) * 128],
                            rhs=xt_sb[ck][:, :, go, :], start=True,
                            stop=True, perf_mode=DR)
                    xsq2 = work.tile([128, 2, NCOL], FP8, tag="xsq2")
                    nc.scalar.activation(
                        out=xsq2,
                        in_=pg[:, :].rearrange("p (b n) -> p b n",
                                               b=2)[:, :, 0:NCOL],
                        func=AF.Square, scale=1.0 / 8.0)
                    nc.tensor.matmul(out=nbp[:, g2 * 512:g2 * 512 + NCOL],
                                     lhsT=ones8, rhs=xsq2, start=True,
                                     stop=False, perf_mode=DR)
                    nc.tensor.matmul(out=nbp[:, g2 * 512:g2 * 512 + NCOL],
                                     lhsT=wb_sb,
                                     rhs=xt_sb[ck][:, :, go, :],
                                     start=False, stop=True, perf_mode=DR)
                    pgs.append(pg)
                rinvb = work.tile([128, 2, NCOL], BF16, tag="rinvb")
                nc.scalar.activation(
                    out=rinvb,
                    in_=nbp[:, :].rearrange("x (b n) -> x b n", b=2)[:, :,
                                                                    0:NCOL],
                    func=AF.Abs_reciprocal_sqrt, scale=0.25, bias=bb_sb)
                for g2 in range(2):
                    g = 2 * gp + g2
                    for mh in range(2):
                        nc.vector.scalar_tensor_tensor(
                            out=pool2[:, mh, g, :],
                            in0=pgs[g2][:, mh * 512:mh * 512 + NCOL],
                            scalar=bc_sb[:, 2 + mh:3 + mh],
                            in1=rinvb[:, g2, :], op0=OP.add, op1=OP.mult)
                if gp in inject:
                    inject[gp]()
                if gp == 3:
                    nc.sync.dma_start(out=pool_out[:, :, 0:8, :],
                                      in_=pool2[:, :, 0:8, :])
            nc.sync.dma_start(out=pool_out[:, :, 8:NG, :],
                              in_=pool2[:, :, 8:NG, :])
    nc.compile()
    return nc


def _build_l2(segments, nt2):
    """segments: tuple of (tile m, p0, p1, pivot g); p0/p1 32-aligned so
    every sub-matmul lands on a valid PE tile position."""
    nc = bacc.Bacc("TRN2", target_bir_lowering=False, debug=False,
                   num_devices=NCORES)
    a_in = nc.declare_dram_parameter("a_in", [128, 2, nt2 * 128], FP8,
                                     isOutput=False)
    pool_in = nc.declare_dram_parameter("pool_in", [128, 2, NG, NCOL], FP8,
                                        isOutput=False)
    t8_out = nc.declare_dram_parameter("t8_out", [128, nt2 * 8], F32,
                                       isOutput=True)
    DR = mybir.MatmulPerfMode.DoubleRow

    with tile.TileContext(nc) as tc:
        with (
            tc.tile_pool(name="singles", bufs=1) as singles,
            tc.tile_pool(name="psA", bufs=6, space="PSUM") as psA,
        ):
            A_sb = singles.tile([128, 2, nt2 * 128], FP8, tag="A")
            nc.sync.dma_start(out=A_sb, in_=a_in[:, :, :])
            pool_sb = singles.tile([128, 2, NG, NCOL], FP8, tag="pool")
            engs = (nc.scalar, nc.gpsimd, nc.scalar, nc.gpsimd)
            for ck in range(4):
                gsl = slice(ck * 4, min((ck + 1) * 4, NG))
                engs[ck].dma_start(out=pool_sb[:, :, gsl, :],
                                   in_=pool_in[:, :, gsl, :])

            t8_sb = singles.tile([128, nt2, 8], F32, tag="t8")
            for m in range(nt2):
                px = psA.tile([128, NCOL], F32, tag="px")
                (sm, p0, p1, g) = segments[m]
                nc.tensor.matmul(
                    out=px,
                    lhsT=A_sb[:, :, m * 128:(m + 1) * 128],
                    rhs=pool_sb[:, :, g - 1, :],
                    start=True, stop=True, perf_mode=DR)
                nc.vector.max(out=t8_sb[:, m, :], in_=px)
            nc.gpsimd.dma_start(
                out=t8_out[:, :],
                in_=t8_sb[:, :, :].rearrange("p t e -> p (t e)"))
    nc.compile()
    return nc


def _build_l3(nt3):
    nc = bacc.Bacc("TRN2", target_bir_lowering=False, debug=False,
                   num_devices=NCORES)
    cand_in = nc.declare_dram_parameter("cand_in", [128, NCORES, nt3, 8], F32,
                                        isOutput=False)
    sc_in = nc.declare_dram_parameter("sc_in", [128, nt3, 4], F32,
                                      isOutput=False)
    hmrg_in = nc.declare_dram_parameter("hmrg_in", [128, nt3, 2], F32,
                                        isOutput=False)
    out2 = nc.declare_dram_parameter("out2", [2, 1], F32, isOutput=True)

    with tile.TileContext(nc) as tc:
        with (
            tc.tile_pool(name="singles", bufs=1) as singles,
            tc.tile_pool(name="work", bufs=2) as work,
            tc.tile_pool(name="psB", bufs=2, space="PSUM") as psB,
        ):
            cand = singles.tile([128, NCORES, nt3, 8], F32, tag="cand")
            nc.sync.dma_start(out=cand, in_=cand_in[:, :, :, :])
            sc_sb = singles.tile([128, nt3, 4], F32, tag="scsb")
            nc.scalar.dma_start(out=sc_sb, in_=sc_in[:, :, :])
            hmrg_sb = singles.tile([128, nt3, 2], F32, tag="hmrg")
            nc.scalar.dma_start(out=hmrg_sb, in_=hmrg_in[:, :, :])
            onesf_sb = singles.tile([128, 1], F32, tag="onesf")
            nc.vector.memset(onesf_sb, 1.0)

            c1v = hmrg_sb[:, :, 0:1].rearrange("p t o -> p o t") \
                .unsqueeze(-1).to_broadcast([128, NCORES, nt3, 8])
            c0v = sc_sb[:, :, 0:1].rearrange("p t o -> p o t") \
                .unsqueeze(-1).to_broadcast([128, NCORES, nt3, 8])
            d_sb = singles.tile([128, NCORES, nt3, 8], F32, tag="dsb")
            nc.vector.tensor_tensor(out=d_sb, in0=cand, in1=c1v, op=OP.mult)
            nc.vector.tensor_tensor(out=d_sb, in0=d_sb, in1=c0v, op=OP.add)
            t8m = singles.tile([128, nt3, 8], F32, tag="t8m")
            for m in range(nt3):
                nc.vector.max(out=t8m[:, m, :], in_=d_sb[:, :, m, :])
            e6 = work.tile([128, nt3, 6], F32, tag="e6")
            nc.scalar.activation(out=e6, in_=t8m[:, :, 0:6], func=AF.Exp)
            se6 = work.tile([128, nt3], F32, tag="se6")
            nc.vector.reduce_sum(out=se6[:, :].unsqueeze(-1), in_=e6,
                                 axis=mybir.AxisListType.X)
            numer = work.tile([128, nt3], F32, tag="numer")
            nc.scalar.activation(out=numer, in_=sc_sb[:, :, 1], func=AF.Exp)
            mx = work.tile([128, nt3], F32, tag="mx")
            nc.vector.tensor_tensor(out=mx[:, :].unsqueeze(-1),
                                    in0=t8m[:, :, 5:6],
                                    in1=sc_sb[:, :, 1:2], op=OP.max)
            em = work.tile([128, nt3], F32, tag="em")
            nc.scalar.activation(out=em, in_=mx, func=AF.Exp)
            deno = work.tile([128, nt3], F32, tag="deno")
            nc.vector.tensor_tensor(out=deno, in0=se6, in1=em, op=OP.subtract)
            nc.vector.tensor_tensor(out=deno, in0=deno, in1=numer, op=OP.add)
            nc.vector.reciprocal(out=deno, in_=deno)
            nc.vector.tensor_tensor(out=deno, in0=deno, in1=numer, op=OP.mult)
            nc.vector.tensor_tensor(out=deno, in0=deno,
                                    in1=hmrg_sb[:, :, 1], op=OP.mult)
            spm = work.tile([128, nt3], F32, tag="spm")
            nc.vector.tensor_tensor(out=spm, in0=sc_sb[:, :, 2],
                                    in1=hmrg_sb[:, :, 1], op=OP.mult)
            pack2 = work.tile([128, 2], F32, tag="pack2")
            nc.vector.reduce_sum(out=pack2[:, 0:1], in_=deno,
                                 axis=mybir.AxisListType.X)
            nc.vector.reduce_sum(out=pack2[:, 1:2], in_=spm,
                                 axis=mybir.AxisListType.X)
            ps_f = psB.tile([2, 1], F32)
            nc.tensor.matmul(out=ps_f, lhsT=pack2, rhs=onesf_sb,
                             start=True, stop=True)
            fin = work.tile([2, 1], F32, tag="fin")
            nc.vector.tensor_scalar(out=fin, in0=ps_f, scalar1=1.0 / NSEQ,
                                    scalar2=None, op0=OP.mult)
            nc.sync.dma_start(out=out2[:, :], in_=fin)
    nc.compile()
    return nc


_NC_CACHE = {}
LAST_RUNS = []


def _hw_runner(nc, in_maps):
    import os
    res = run_bass_kernel_spmd(
        nc, in_maps, list(range(NCORES)),
        trace=bool(os.environ.get("KERNEL_TRACE")))
    LAST_RUNS.append(res)
    return res.results


def _get(name, builder):
    if name not in _NC_CACHE:
        _NC_CACHE[name] = builder()
    return _NC_CACHE[name]


def kernel(frame_embeds, other_frame_embeds, W, b, bridge, _runner=None):
    frame_embeds = np.asarray(frame_embeds, dtype=np.float32)
    other_frame_embeds = np.asarray(other_frame_embeds, dtype=np.float32)
    W = np.asarray(W, dtype=np.float32)
    b = np.asarray(b, dtype=np.float32)
    bridge = np.asarray(bridge, dtype=np.int32)
    runner = _runner if _runner is not None else _hw_runner

    # ---- host-side sharding / layout (indexing + dtype cast only) ----
    fe_seq = frame_embeds.transpose(0, 2, 1, 3).reshape(NSEQ, T, HID)
    ofe_seq = other_frame_embeds.transpose(0, 2, 1, 3).reshape(NSEQ, T, HID)
    perm = np.argsort(bridge[:, 1], kind="stable")
    fe_sorted = fe_seq[perm]
    bridge_s = bridge[perm].astype(np.float32)

    bh, bp, bt = bridge_s[:, 0], bridge_s[:, 1], bridge_s[:, 2]
    alpha = (bp - bh) / (bt - bh)
    sigma = alpha * (bt - bp)
    c1 = 1.0 / (sigma * sigma)
    piv = bridge[perm][:, 1].astype(np.int64)

    NP8 = mybir.dt.np(FP8)
    # W scaled x16 so fp8 entries sit in normal range; [p, kh, c] layout
    w_host = np.ascontiguousarray(
        (16.0 * W).reshape(2, 128, PROJ).transpose(1, 0, 2).astype(NP8))
    bb = float((b * b).sum())
    b2 = b.reshape(2, 128).T                        # [p, mh]
    bc = np.ascontiguousarray(np.concatenate(
        [b2, 16.0 * b2, np.full((128, 1), bb)], -1).astype(np.float32))
    wb8 = (8.0 * (W @ b)).reshape(2, 128)          # [kh, p]
    wb_host = np.ascontiguousarray(
        np.broadcast_to(wb8.T[:, :, None], (128, 2, 128)).astype(NP8))

    in1 = []
    for k in range(NCORES):
        sl = slice(k * SPC, (k + 1) * SPC)
        cur = fe_sorted[sl]                      # (200, 16, 256)
        oth = ofe_seq[sl]                        # (200, 16, 256)
        both_int = np.concatenate([cur[:, 1:T - 1, :], oth[:, 1:T - 1, :]],
                                  axis=0)        # (400, 14, 256)
        # [hid, g, col] -> [p, kh, g, col]
        xt_int = np.ascontiguousarray(
            both_int.transpose(2, 1, 0).reshape(2, 128, NG, NCOL)
            .transpose(1, 0, 2, 3).astype(NP8))
        g1 = cur[np.arange(SPC), piv[sl], :]     # (200, 256)
        g_cols = np.concatenate([cur[:, 0, :], g1, cur[:, T - 1, :]], axis=0)
        xt_gh = np.ascontiguousarray(
            g_cols.T.reshape(2, 128, GSZ).transpose(1, 0, 2).astype(NP8))
        hrow = np.concatenate([
            1.0 - alpha[sl], alpha[sl], -0.5 * c1[sl]]).reshape(1, 3 * SPC)
        in1.append({
            "xt_int": xt_int, "xt_g": xt_gh, "w_in": w_host, "bc_in": bc,
            "wb_in": wb_host,
            "hrow_in": np.ascontiguousarray(hrow.astype(np.float32)),
        })

    nc1 = _get("l1", _build_l1)
    r1 = runner(nc1, in1)

    # ---- host: slot layout (pivot groups padded to full 128-row tiles,
    # so every cross tile is a single full DoubleRow matmul) ----
    counts = np.bincount(piv, minlength=T - 1)[1:T - 1]      # g = 1..14
    caps = ((counts + 127) // 128) * 128
    starts = np.zeros(NG, np.int64)
    starts[1:] = np.cumsum(caps)[:-1]
    nslots = int(caps.sum())
    nt2 = nslots // 128
    npad2 = nslots
    rank = np.arange(NSEQ, dtype=np.int64) - np.concatenate(
        [[0], np.cumsum(counts)])[:-1][piv - 1]
    slot_of = starts[piv - 1] + rank                        # sorted row -> slot

    tile_g = np.zeros(nt2, np.int64)
    for g in range(NG):
        tile_g[starts[g] // 128:(starts[g] + caps[g]) // 128] = g + 1
    segments = tuple((m, 0, 128, int(tile_g[m])) for m in range(nt2))

    # A [128, 2, npad2] fp8: scatter a-vectors to their slots
    a_all = np.concatenate([r1[k]["a_out"] for k in range(NCORES)], axis=2)
    a_pad = np.zeros((128, 2, npad2), dtype=NP8)
    a_pad[:, :, slot_of] = a_all
    a_pad = np.ascontiguousarray(a_pad)

    in2 = [{"a_in": a_pad, "pool_in": r1[k]["pool_out"]}
           for k in range(NCORES)]
    key = ("l2", segments, nt2)
    if key not in _NC_CACHE:
        _NC_CACHE[key] = _build_l2(segments, nt2)
    r2 = runner(_NC_CACHE[key], in2)

    # ---- host: gather top8 + scalars into merge layout (indexing) ----
    nt3 = max(16, nt2)
    npad3 = nt3 * 128
    cand = np.zeros((128, NCORES, nt3, 8), np.float32)
    for k in range(NCORES):
        cand[:, k, :nt2, :] = r2[k]["t8_out"].reshape(128, nt2, 8)
    cand = np.ascontiguousarray(cand)
    sc_slots = np.zeros((npad3, 4), np.float32)
    sc_all = np.concatenate(
        [r1[k]["sc_out"].reshape(SPC, 4) for k in range(NCORES)], axis=0)
    sc_slots[slot_of] = sc_all
    sc_in = np.ascontiguousarray(
        sc_slots.reshape(nt3, 128, 4).transpose(1, 0, 2))

    c1_pad = np.zeros(npad3, np.float32)
    c1_pad[slot_of] = c1 / 256.0
    mask_pad = np.zeros(npad3, np.float32)
    mask_pad[slot_of] = 1.0
    hmrg = np.ascontiguousarray(
        np.stack([c1_pad, mask_pad], -1).reshape(nt3, 128, 2)
        .transpose(1, 0, 2))

    in3 = [{"cand_in": cand, "sc_in": sc_in, "hmrg_in": hmrg}
           for _ in range(NCORES)]
    key3 = ("l3", nt3)
    if key3 not in _NC_CACHE:
        _NC_CACHE[key3] = _build_l3(nt3)
    r3 = runner(_NC_CACHE[key3], in3)

    out = r3[0]["out2"]
    return (np.asarray(np.float32(out[0, 0])), np.asarray(np.float32(out[1, 0])))


# revision 26
# speedup vs baseline: 1.3770x; 1.2228x over previous
"""Brownian-bridge criterion loss on 8 Trainium2 NeuronCores (3 launches).

Data-parallel over the 1600 pivot-sorted cur sequences (200/core); the
negative pool is column-sharded (each core scores all 1664 row-slots against
its own 400 sequences' pivot frames). Host work between launches is pure
indexing (concat / transpose / gather).

Launch 1 (value-independent): transposed projection ([hid, cols] inputs,
  W stationary) of the 400 seqs' interior frames; column norms via all-ones
  [128,128] stationary matmul (sums broadcast across PSUM partitions) +
  1/sqrt on scalar engine; bias+normalize fused into PSUM evacuation ->
  pool [c, 14, 400] bf16. Small projection of cur head/pivot/tail columns
  -> a-vectors [c, 200], per-row dots via elementwise+ones-matmul, per-row
  scalars (c0, s, softplus) packed [200, 4].

Launch 2 (specialized to the bridge pivot multiset): cross = A^T @ pool_g
  per (row-tile, pivot-run) segment, Max8 per tile -> top8 [128, 13, 8].

Launch 3 (value-independent): merge the 8 cores' top8 (host-gathered into
  [128, 8, 13, 8]), dist = c1*cross + c0, top-8 of 64, exp /
  top-5-excluding-self trick, masked mean -> (brownian, head_tail).
"""

import sys

sys.path.insert(0, "/opt/trn_rl_repo")

import numpy as np
import ml_dtypes

import concourse.bacc as bacc
import concourse.bass as bass
import concourse.mybir as mybir
import concourse.tile as tile
from concourse.bass_utils import run_bass_kernel_spmd

F32 = mybir.dt.float32
FP8 = mybir.dt.float8e4
BF16 = mybir.dt.bfloat16
I32 = mybir.dt.int32
AF = mybir.ActivationFunctionType
OP = mybir.AluOpType

BS, T, Q, HID, PROJ = 16, 16, 100, 256, 256
NSEQ = BS * Q              # 1600 positive sequences
NCORES = 8
SPC = NSEQ // NCORES       # 200 cur sequences per core
NG = T - 2                 # 14 interior pivot positions (1..14)
NCOL = 2 * SPC             # 400 negative-pool columns per core
NT = (NSEQ + 127) // 128   # 13 row tiles
NPAD = NT * 128            # 1664
DELTA = 0.3
GSZ = 3 * SPC              # 600 head/pivot/tail columns
NT3 = 16                   # fixed slot-tile count for launch 3 (2048 slots)


def _build_l1():
    nc = bacc.Bacc("TRN2", target_bir_lowering=False, debug=False,
                   num_devices=NCORES)
    # fp8 inputs, k-halves paired in dim 1 for DoubleRow matmuls.
    # xt carries raw frames; W is pre-scaled by 16 on the host so fp8 stays
    # in normal range (embeddings come out 16x; norms/dots rescale below).
    xt_int = nc.declare_dram_parameter("xt_int", [128, 2, NG, NCOL], FP8,
                                       isOutput=False)
    xt_g = nc.declare_dram_parameter("xt_g", [128, 2, GSZ], FP8,
                                     isOutput=False)
    w_in = nc.declare_dram_parameter("w_in", [128, 2, PROJ], FP8,
                                     isOutput=False)
    # bc cols: 0,1 = b (mh halves); 2,3 = 16b; 4 = ||b||^2 (replicated)
    bc_in = nc.declare_dram_parameter("bc_in", [128, 5], F32, isOutput=False)
    # [1,200] host scalars: (1-alpha), alpha, -1/(2 sigma^2)
    hrow_in = nc.declare_dram_parameter("hrow_in", [1, 3 * SPC], F32,
                                        isOutput=False)
    # pool/a leave as fp8 scaled 16x (unit-norm embeddings x16)
    pool_out = nc.declare_dram_parameter("pool_out", [128, 2, NG, NCOL], FP8,
                                         isOutput=True)
    a_out = nc.declare_dram_parameter("a_out", [128, 2, SPC], FP8,
                                      isOutput=True)
    sc_out = nc.declare_dram_parameter("sc_out", [1, SPC * 4], F32,
                                       isOutput=True)
    DR = mybir.MatmulPerfMode.DoubleRow

    with tile.TileContext(nc) as tc:
        with (
            tc.tile_pool(name="singles", bufs=1) as singles,
            tc.tile_pool(name="work", bufs=4) as work,
            tc.tile_pool(name="psA", bufs=6, space="PSUM") as psA,
            tc.tile_pool(name="psB", bufs=1, space="PSUM") as psB,
        ):
            w_sb = singles.tile([128, 2, PROJ], FP8, tag="w")
            nc.sync.dma_start(out=w_sb, in_=w_in[:, :, :])
            bc_sb = singles.tile([128, 5], F32, tag="bc")
            nc.gpsimd.dma_start(out=bc_sb, in_=bc_in[:, :])
            bias_sb = bc_sb[:, 0:2]
            bb_sb = bc_sb[:, 4:5]
            hrow_sb = singles.tile([1, 3 * SPC], F32, tag="hrow")
            nc.gpsimd.dma_start(out=hrow_sb, in_=hrow_in[:, :])
            h_oma = hrow_sb[:, 0:SPC]
            h_alp = hrow_sb[:, SPC:2 * SPC]
            h_c1hneg = hrow_sb[:, 2 * SPC:3 * SPC]
            xtg_sb = singles.tile([128, 2, GSZ], FP8, tag="xtg")
            nc.sync.dma_start(out=xtg_sb, in_=xt_g[:, :, :])
            NCHK = 2
            GPC = NG // NCHK  # 7
            xt_sb = [None] * NCHK
            for ck in range(NCHK):
                t_x = singles.tile([128, 2, GPC, NCOL], FP8, tag=f"xt{ck}")
                eng = nc.sync if ck == 0 else nc.gpsimd
                eng.dma_start(out=t_x,
                              in_=xt_int[:, :, ck * GPC:(ck + 1) * GPC, :])
                xt_sb[ck] = t_x

            ones8 = singles.tile([128, 2, 128], FP8, tag="ones8")
            nc.vector.memset(ones8, 1.0)
            ones16row = singles.tile([1, 128], BF16, tag="ones16row")
            nc.vector.memset(ones16row, 16.0)

            pool2 = singles.tile([128, 2, NG, NCOL], FP8, tag="pool2")

            # --- g-cols projection first (only needs xtg + W) ---
            u_g = [work.tile([128, GSZ], F32, tag=f"ug{mh}", name=f"ug{mh}")
                   for mh in range(2)]
            xsq_g = work.tile([128, 2, GSZ], FP8, tag="xsqg")
            for mh in range(2):
                for c2 in range(2):
                    sl = slice(c2 * 300, (c2 + 1) * 300)
                    p = psA.tile([128, 300], F32, tag="p", bufs=6,
                                 name="pgg")
                    nc.tensor.matmul(
                        out=p, lhsT=w_sb[:, :, mh * 128:(mh + 1) * 128],
                        rhs=xtg_sb[:, :, sl], start=True, stop=True,
                        perf_mode=DR)
                    # u_g = ps/16 + b  (original embedding scale)
                    nc.vector.tensor_scalar(
                        out=u_g[mh][:, sl], in0=p, scalar1=1.0 / 16.0,
                        scalar2=bias_sb[:, mh:mh + 1], op0=OP.mult,
                        op1=OP.add)
                    nc.scalar.activation(out=xsq_g[:, mh, sl], in_=p,
                                         func=AF.Square, scale=1.0 / 16.0,
                                         bias=bias_sb[:, mh:mh + 1])

            # --- main loop with the g-phase tail interleaved ---
            rinv_g = []
            a_f = [work.tile([128, SPC], F32, tag=f"af{mh}", name=f"af{mh}")
                   for mh in range(2)]
            a_bf = work.tile([128, 2, SPC], FP8, tag="abf")
            pst2 = work.tile([128, 2, 3, SPC], FP8, tag="pst2")
            pd = []
            ps_s = None

            def emit_gn():
                for j in range(3):
                    gn = psB.tile([1, SPC], F32, name=f"gn{j}", tag="acc1",
                                  bufs=1)
                    nc.tensor.matmul(
                        out=gn, lhsT=ones8[:, :, 0:1],
                        rhs=xsq_g[:, :, j * SPC:(j + 1) * SPC],
                        start=True, stop=True, perf_mode=DR)
                    rv = work.tile([1, SPC], F32, tag=f"rinvg{j}",
                                   name=f"rinvg{j}")
                    nc.scalar.activation(out=rv, in_=gn,
                                         func=AF.Abs_reciprocal_sqrt)
                    rinv_g.append(rv)

            def emit_pss():
                nonlocal ps_s
                # s02 = (1-a)/||u0||, a/||u2||; broadcast x16 via the
                # ones16 stationary -> a-vectors come out 16x (fp8-normal)
                s02 = work.tile([1, 2 * SPC], BF16, tag="s02")
                nc.vector.tensor_tensor(out=s02[:, 0:SPC], in0=h_oma,
                                        in1=rinv_g[0], op=OP.mult)
                nc.vector.tensor_tensor(out=s02[:, SPC:2 * SPC], in0=h_alp,
                                        in1=rinv_g[2], op=OP.mult)
                ps_s = psB.tile([128, 2 * SPC], F32, tag="acc1", bufs=1)
                nc.tensor.matmul(out=ps_s, lhsT=ones16row, rhs=s02,
                                 start=True, stop=True)

            def emit_abuild():
                for mh in range(2):
                    u0 = u_g[mh][:, 0:SPC]
                    u1 = u_g[mh][:, SPC:2 * SPC]
                    u2 = u_g[mh][:, 2 * SPC:3 * SPC]
                    t1 = work.tile([128, SPC], F32, tag="t1")
                    nc.vector.tensor_tensor(out=t1, in0=u0,
                                            in1=ps_s[:, 0:SPC], op=OP.mult)
                    t2 = work.tile([128, SPC], F32, tag="t2")
                    nc.vector.tensor_tensor(out=t2, in0=u2,
                                            in1=ps_s[:, SPC:2 * SPC],
                                            op=OP.mult)
                    nc.vector.tensor_tensor(out=a_f[mh], in0=t1, in1=t2,
                                            op=OP.add)
                    nc.vector.tensor_copy(out=a_bf[:, mh, :], in_=a_f[mh])
                    nc.gpsimd.tensor_tensor(out=pst2[:, mh, 0, :],
                                            in0=a_f[mh], in1=u1, op=OP.mult)
                    nc.gpsimd.tensor_tensor(out=pst2[:, mh, 1, :],
                                            in0=a_f[mh], in1=a_f[mh],
                                            op=OP.mult)
                    nc.gpsimd.tensor_tensor(out=pst2[:, mh, 2, :], in0=u0,
                                            in1=u2, op=OP.mult)
                nc.gpsimd.dma_start(out=a_out[:, :, :], in_=a_bf)

            def emit_pd():
                for j in range(3):
                    pdj = psB.tile([1, SPC], F32, name=f"pd{j}", tag="acc1",
                                   bufs=1)
                    nc.tensor.matmul(
                        out=pdj, lhsT=ones8[:, :, 0:1],
                        rhs=pst2[:, :, j, :], start=True, stop=True,
                        perf_mode=DR)
                    pd.append(pdj)

            def emit_scmath():
                # a' = 16a: pd0 = 16(a.u1), pd1 = 256(a.a), pd2 = u0.u2
                sc_pack = work.tile([1, SPC, 4], F32, tag="scpack")
                nc.vector.memset(sc_pack[:, :, 3:4], 0.0)
                q_r = work.tile([1, SPC], F32, tag="qr")
                nc.vector.tensor_tensor(out=q_r, in0=pd[0], in1=rinv_g[1],
                                        op=OP.mult)
                aa_r = work.tile([1, SPC], F32, tag="aar")
                nc.vector.tensor_scalar(out=aa_r, in0=pd[1],
                                        scalar1=1.0 / 256.0, scalar2=None,
                                        op0=OP.mult)
                sc_r = work.tile([1, SPC], F32, tag="scr")
                nc.vector.tensor_tensor(out=sc_r, in0=pd[2], in1=rinv_g[0],
                                        op=OP.mult)
                nc.vector.tensor_tensor(out=sc_r, in0=sc_r, in1=rinv_g[2],
                                        op=OP.mult)
                # c0 = (1 + aa) * (-1/(2 sigma^2))
                nc.vector.scalar_tensor_tensor(
                    out=sc_pack[:, :, 0], in0=aa_r, scalar=1.0,
                    in1=h_c1hneg, op0=OP.add, op1=OP.mult)
                # s = (1 - 2q + aa) * (-1/(2 sigma^2)); q = q_r/16
                t_r = work.tile([1, SPC], F32, tag="tr")
                nc.vector.scalar_tensor_tensor(
                    out=t_r, in0=q_r, scalar=-0.125, in1=aa_r,
                    op0=OP.mult, op1=OP.add)
                nc.vector.tensor_scalar(out=t_r, in0=t_r, scalar1=1.0,
                                        scalar2=None, op0=OP.add)
                nc.vector.tensor_tensor(out=sc_pack[:, :, 1], in0=t_r,
                                        in1=h_c1hneg, op=OP.mult)
                # raw score; softplus(delta - score) happens in launch 3
                nc.vector.tensor_copy(out=sc_pack[:, :, 2], in_=sc_r)
                nc.gpsimd.dma_start(
                    out=sc_out[:, :],
                    in_=sc_pack[:, :, :].rearrange("o s q -> o (s q)"))

            inject = {1: emit_gn, 2: emit_pss, 3: emit_abuild, 5: emit_pd,
                      6: emit_scmath}
            for gp in range(NG // 2):
                for g2 in range(2):
                    g = 2 * gp + g2
                    ck, go = g // GPC, g % GPC
                    ps = [None, None]
                    for mh in range(2):
                        p = psA.tile([128, NCOL], F32, tag="p", bufs=6,
                                     name=f"p{g2}{mh}")
                        nc.tensor.matmul(
                            out=p, lhsT=w_sb[:, :, mh * 128:(mh + 1) * 128],
                            rhs=xt_sb[ck][:, :, go, :], start=True,
                            stop=True, perf_mode=DR)
                        ps[mh] = p
                    xsq2 = work.tile([128, 2, NCOL], FP8, tag="xsq2")
                    for mh in range(2):
                        nc.scalar.activation(out=xsq2[:, mh, :], in_=ps[mh],
                                             func=AF.Square, scale=1.0 / 16.0,
                                             bias=bias_sb[:, mh:mh + 1])
                    nb = psB.tile([128, NCOL], F32, tag="accb", bufs=1)
                    nc.tensor.matmul(out=nb, lhsT=ones8, rhs=xsq2,
                                     start=True, stop=True, perf_mode=DR)
                    rinvb = work.tile([128, NCOL], BF16, tag="rinvb")
                    nc.scalar.activation(out=rinvb, in_=nb,
                                         func=AF.Abs_reciprocal_sqrt)
                    for mh in range(2):
                        nc.vector.scalar_tensor_tensor(
                            out=pool2[:, mh, g, :], in0=ps[mh],
                            scalar=bc_sb[:, 2 + mh:3 + mh],
                            in1=rinvb, op0=OP.add, op1=OP.mult)
                if gp in inject:
                    inject[gp]()
                if gp == 3:
                    nc.sync.dma_start(out=pool_out[:, :, 0:8, :],
                                      in_=pool2[:, :, 0:8, :])
            nc.sync.dma_start(out=pool_out[:, :, 8:NG, :],
                              in_=pool2[:, :, 8:NG, :])
    nc.compile()
    return nc


def _build_l2(segments, nt2):
    """segments: tuple of (tile m, p0, p1, pivot g); p0/p1 32-aligned so
    every sub-matmul lands on a valid PE tile position."""
    nc = bacc.Bacc("TRN2", target_bir_lowering=False, debug=False,
                   num_devices=NCORES)
    a_in = nc.declare_dram_parameter("a_in", [128, 2, nt2 * 128], FP8,
                                     isOutput=False)
    pool_in = nc.declare_dram_parameter("pool_in", [128, 2, NG, NCOL], FP8,
                                        isOutput=False)
    t8_out = nc.declare_dram_parameter("t8_out", [128, nt2 * 8], F32,
                                       isOutput=True)
    DR = mybir.MatmulPerfMode.DoubleRow

    with tile.TileContext(nc) as tc:
        with (
            tc.tile_pool(name="singles", bufs=1) as singles,
            tc.tile_pool(name="psA", bufs=6, space="PSUM") as psA,
        ):
            A_sb = singles.tile([128, 2, nt2 * 128], FP8, tag="A")
            nc.sync.dma_start(out=A_sb, in_=a_in[:, :, :])
            pool_sb = singles.tile([128, 2, NG, NCOL], FP8, tag="pool")
            engs = (nc.scalar, nc.gpsimd, nc.scalar, nc.gpsimd)
            for ck in range(4):
                gsl = slice(ck * 4, min((ck + 1) * 4, NG))
                engs[ck].dma_start(out=pool_sb[:, :, gsl, :],
                                   in_=pool_in[:, :, gsl, :])

            t8_sb = singles.tile([128, nt2, 8], F32, tag="t8")
            for m in range(nt2):
                px = psA.tile([128, NCOL], F32, tag="px")
                (sm, p0, p1, g) = segments[m]
                nc.tensor.matmul(
                    out=px,
                    lhsT=A_sb[:, :, m * 128:(m + 1) * 128],
                    rhs=pool_sb[:, :, g - 1, :],
                    start=True, stop=True, perf_mode=DR)
                nc.vector.max(out=t8_sb[:, m, :], in_=px)
            nc.gpsimd.dma_start(
                out=t8_out[:, :],
                in_=t8_sb[:, :, :].rearrange("p t e -> p (t e)"))
    nc.compile()
    return nc


def _build_l3(nt3):
    nc = bacc.Bacc("TRN2", target_bir_lowering=False, debug=False,
                   num_devices=NCORES)
    cand_in = nc.declare_dram_parameter("cand_in", [128, NCORES, nt3, 8], F32,
                                        isOutput=False)
    sc_in = nc.declare_dram_parameter("sc_in", [128, nt3, 4], F32,
                                      isOutput=False)
    hmrg_in = nc.declare_dram_parameter("hmrg_in", [128, nt3, 2], F32,
                                        isOutput=False)
    out2 = nc.declare_dram_parameter("out2", [2, 1], F32, isOutput=True)

    with tile.TileContext(nc) as tc:
        with (
            tc.tile_pool(name="singles", bufs=1) as singles,
            tc.tile_pool(name="work", bufs=2) as work,
            tc.tile_pool(name="psB", bufs=2, space="PSUM") as psB,
        ):
            cand = singles.tile([128, NCORES, nt3, 8], F32, tag="cand")
            nc.sync.dma_start(out=cand, in_=cand_in[:, :, :, :])
            sc_sb = singles.tile([128, nt3, 4], F32, tag="scsb")
            nc.scalar.dma_start(out=sc_sb, in_=sc_in[:, :, :])
            hmrg_sb = singles.tile([128, nt3, 2], F32, tag="hmrg")
            nc.scalar.dma_start(out=hmrg_sb, in_=hmrg_in[:, :, :])
            onesf_sb = singles.tile([128, 1], F32, tag="onesf")
            nc.vector.memset(onesf_sb, 1.0)

            c1v = hmrg_sb[:, :, 0:1].rearrange("p t o -> p o t") \
                .unsqueeze(-1).to_broadcast([128, NCORES, nt3, 8])
            c0v = sc_sb[:, :, 0:1].rearrange("p t o -> p o t") \
                .unsqueeze(-1).to_broadcast([128, NCORES, nt3, 8])
            d_sb = singles.tile([128, NCORES, nt3, 8], F32, tag="dsb")
            nc.vector.tensor_tensor(out=d_sb, in0=cand, in1=c1v, op=OP.mult)
            nc.vector.tensor_tensor(out=d_sb, in0=d_sb, in1=c0v, op=OP.add)
            t8m = singles.tile([128, nt3, 8], F32, tag="t8m")
            for m in range(nt3):
                nc.vector.max(out=t8m[:, m, :], in_=d_sb[:, :, m, :])
            e6 = work.tile([128, nt3, 6], F32, tag="e6")
            nc.scalar.activation(out=e6, in_=t8m[:, :, 0:6], func=AF.Exp)
            se6 = work.tile([128, nt3], F32, tag="se6")
            nc.vector.reduce_sum(out=se6[:, :].unsqueeze(-1), in_=e6,
                                 axis=mybir.AxisListType.X)
            numer = work.tile([128, nt3], F32, tag="numer")
            nc.scalar.activation(out=numer, in_=sc_sb[:, :, 1], func=AF.Exp)
            mx = work.tile([128, nt3], F32, tag="mx")
            nc.vector.tensor_tensor(out=mx[:, :].unsqueeze(-1),
                                    in0=t8m[:, :, 5:6],
                                    in1=sc_sb[:, :, 1:2], op=OP.max)
            em = work.tile([128, nt3], F32, tag="em")
            nc.scalar.activation(out=em, in_=mx, func=AF.Exp)
            deno = work.tile([128, nt3], F32, tag="deno")
            nc.vector.tensor_tensor(out=deno, in0=se6, in1=em, op=OP.subtract)
            nc.vector.tensor_tensor(out=deno, in0=deno, in1=numer, op=OP.add)
            nc.vector.reciprocal(out=deno, in_=deno)
            nc.vector.tensor_tensor(out=deno, in0=deno, in1=numer, op=OP.mult)
            nc.vector.tensor_tensor(out=deno, in0=deno,
                                    in1=hmrg_sb[:, :, 1], op=OP.mult)
            # softplus(delta - score) from the raw score column
            delta_sb = work.tile([128, 1], F32, tag="delta")
            nc.vector.memset(delta_sb, DELTA)
            e_sp = work.tile([128, nt3], F32, tag="esp")
            nc.scalar.activation(out=e_sp, in_=sc_sb[:, :, 2], func=AF.Exp,
                                 scale=-1.0, bias=delta_sb)
            nc.scalar.activation(out=e_sp, in_=e_sp, func=AF.Ln, bias=1.0)
            spm = work.tile([128, nt3], F32, tag="spm")
            nc.vector.tensor_tensor(out=spm, in0=e_sp,
                                    in1=hmrg_sb[:, :, 1], op=OP.mult)
            pack2 = work.tile([128, 2], F32, tag="pack2")
            nc.vector.reduce_sum(out=pack2[:, 0:1], in_=deno,
                                 axis=mybir.AxisListType.X)
            nc.vector.reduce_sum(out=pack2[:, 1:2], in_=spm,
                                 axis=mybir.AxisListType.X)
            ps_f = psB.tile([2, 1], F32)
            nc.tensor.matmul(out=ps_f, lhsT=pack2, rhs=onesf_sb,
                             start=True, stop=True)
            fin = work.tile([2, 1], F32, tag="fin")
            nc.vector.tensor_scalar(out=fin, in0=ps_f, scalar1=1.0 / NSEQ,
                                    scalar2=None, op0=OP.mult)
            nc.sync.dma_start(out=out2[:, :], in_=fin)
    nc.compile()
    return nc


_NC_CACHE = {}
LAST_RUNS = []


def _hw_runner(nc, in_maps):
    import os
    res = run_bass_kernel_spmd(
        nc, in_maps, list(range(NCORES)),
        trace=bool(os.environ.get("KERNEL_TRACE")))
    LAST_RUNS.append(res)
    return res.results


def _get(name, builder):
    if name not in _NC_CACHE:
        _NC_CACHE[name] = builder()
    return _NC_CACHE[name]


def kernel(frame_embeds, other_frame_embeds, W, b, bridge, _runner=None):
    frame_embeds = np.asarray(frame_embeds, dtype=np.float32)
    other_frame_embeds = np.asarray(other_frame_embeds, dtype=np.float32)
    W = np.asarray(W, dtype=np.float32)
    b = np.asarray(b, dtype=np.float32)
    bridge = np.asarray(bridge, dtype=np.int32)
    runner = _runner if _runner is not None else _hw_runner

    # ---- host-side sharding / layout (indexing + dtype cast only) ----
    fe_seq = frame_embeds.transpose(0, 2, 1, 3).reshape(NSEQ, T, HID)
    ofe_seq = other_frame_embeds.transpose(0, 2, 1, 3).reshape(NSEQ, T, HID)
    perm = np.argsort(bridge[:, 1], kind="stable")
    fe_sorted = fe_seq[perm]
    bridge_s = bridge[perm].astype(np.float32)

    bh, bp, bt = bridge_s[:, 0], bridge_s[:, 1], bridge_s[:, 2]
    alpha = (bp - bh) / (bt - bh)
    sigma = alpha * (bt - bp)
    c1 = 1.0 / (sigma * sigma)
    piv = bridge[perm][:, 1].astype(np.int64)

    NP8 = mybir.dt.np(FP8)
    # W scaled x16 so fp8 entries sit in normal range; [p, kh, c] layout
    w_host = np.ascontiguousarray(
        (16.0 * W).reshape(2, 128, PROJ).transpose(1, 0, 2).astype(NP8))
    bb = float((b * b).sum())
    b2 = b.reshape(2, 128).T                        # [p, mh]
    bc = np.ascontiguousarray(np.concatenate(
        [b2, 16.0 * b2, np.full((128, 1), bb)], -1).astype(np.float32))

    in1 = []
    for k in range(NCORES):
        sl = slice(k * SPC, (k + 1) * SPC)
        cur = fe_sorted[sl]                      # (200, 16, 256)
        oth = ofe_seq[sl]                        # (200, 16, 256)
        both_int = np.concatenate([cur[:, 1:T - 1, :], oth[:, 1:T - 1, :]],
                                  axis=0)        # (400, 14, 256)
        # [hid, g, col] -> [p, kh, g, col]
        xt_int = np.ascontiguousarray(
            both_int.transpose(2, 1, 0).reshape(2, 128, NG, NCOL)
            .transpose(1, 0, 2, 3).astype(NP8))
        g1 = cur[np.arange(SPC), piv[sl], :]     # (200, 256)
        g_cols = np.concatenate([cur[:, 0, :], g1, cur[:, T - 1, :]], axis=0)
        xt_gh = np.ascontiguousarray(
            g_cols.T.reshape(2, 128, GSZ).transpose(1, 0, 2).astype(NP8))
        hrow = np.concatenate([
            1.0 - alpha[sl], alpha[sl], -0.5 * c1[sl]]).reshape(1, 3 * SPC)
        in1.append({
            "xt_int": xt_int, "xt_g": xt_gh, "w_in": w_host, "bc_in": bc,
            "hrow_in": np.ascontiguousarray(hrow.astype(np.float32)),
        })

    nc1 = _get("l1", _build_l1)
    r1 = runner(nc1, in1)

    # ---- host: slot layout (pivot groups padded to full 128-row tiles,
    # so every cross tile is a single full DoubleRow matmul) ----
    counts = np.bincount(piv, minlength=T - 1)[1:T - 1]      # g = 1..14
    caps = ((counts + 127) // 128) * 128
    starts = np.zeros(NG, np.int64)
    starts[1:] = np.cumsum(caps)[:-1]
    nslots = int(caps.sum())
    nt2 = nslots // 128
    npad2 = nslots
    rank = np.arange(NSEQ, dtype=np.int64) - np.concatenate(
        [[0], np.cumsum(counts)])[:-1][piv - 1]
    slot_of = starts[piv - 1] + rank                        # sorted row -> slot

    tile_g = np.zeros(nt2, np.int64)
    for g in range(NG):
        tile_g[starts[g] // 128:(starts[g] + caps[g]) // 128] = g + 1
    segments = tuple((m, 0, 128, int(tile_g[m])) for m in range(nt2))

    # A [128, 2, npad2] fp8: scatter a-vectors to their slots
    a_all = np.concatenate([r1[k]["a_out"] for k in range(NCORES)], axis=2)
    a_pad = np.zeros((128, 2, npad2), dtype=NP8)
    a_pad[:, :, slot_of] = a_all
    a_pad = np.ascontiguousarray(a_pad)

    in2 = [{"a_in": a_pad, "pool_in": r1[k]["pool_out"]}
           for k in range(NCORES)]
    key = ("l2", segments, nt2)
    if key not in _NC_CACHE:
        _NC_CACHE[key] = _build_l2(segments, nt2)
    r2 = runner(_NC_CACHE[key], in2)

    # ---- host: gather top8 + scalars into merge layout (indexing) ----
    nt3 = max(16, nt2)
    npad3 = nt3 * 128
    cand = np.zeros((128, NCORES, nt3, 8), np.float32)
    for k in range(NCORES):
        cand[:, k, :nt2, :] = r2[k]["t8_out"].reshape(128, nt2, 8)
    cand = np.ascontiguousarray(cand)
    sc_slots = np.zeros((npad3, 4), np.float32)
    sc_all = np.concatenate(
        [r1[k]["sc_out"].reshape(SPC, 4) for k in range(NCORES)], axis=0)
    sc_slots[slot_of] = sc_all
    sc_in = np.ascontiguousarray(
        sc_slots.reshape(nt3, 128, 4).transpose(1, 0, 2))

    c1_pad = np.zeros(npad3, np.float32)
    c1_pad[slot_of] = c1 / 256.0
    mask_pad = np.zeros(npad3, np.float32)
    mask_pad[slot_of] = 1.0
    hmrg = np.ascontiguousarray(
        np.stack([c1_pad, mask_pad], -1).reshape(nt3, 128, 2)
        .transpose(1, 0, 2))

    in3 = [{"cand_in": cand, "sc_in": sc_in, "hmrg_in": hmrg}
           for _ in range(NCORES)]
    key3 = ("l3", nt3)
    if key3 not in _NC_CACHE:
        _NC_CACHE[key3] = _build_l3(nt3)
    r3 = runner(_NC_CACHE[key3], in3)

    out = r3[0]["out2"]
    return (np.asarray(np.float32(out[0, 0])), np.asarray(np.float32(out[1, 0])))
